# revision 44
# baseline (speedup 1.0000x reference)
"""Trainium2 Bass kernel for nn_AttnBlock_12704513262242.

Math (per sample b, W=2048 "positions" with scalar q/k values):
  h   = layernorm(x) * gamma + beta
  q,k,v = h @ W* + b*
  attn  = softmax(-|q_j - k_i|, over i)
  h2[j] = sum_i attn[j,i] * v[i]
  out   = x + h2 @ Wp + bp

Sharding: feature-parallel QKV/proj (each core owns a 256-col slice of all
four weight matrices, host-cast to fp16), AllToAll to redistribute q/k/v
sample-major, data-parallel attention (4 samples per core), AllGather of
h2, feature-sliced output projection.  Host concatenates 8 [32,256] slices.

Default mode "v2" (the fast path; "naive"/"binned" are older fallbacks):
  * LayerNorm is deferred through the QKV matmul: matmul raw x.T (starts
    immediately, warms the PE pstate), stats via a selector matmul on a
    [128,512] view + Newton rsqrt on DVE (no ACT table switches), then
    qkv = rstd*(x@W + mu*(-colsum W) + std*b) via one K=2 rank-2 matmul
    and a scaling psum->sbuf copy.
  * Binned softmin attention: exp(-|q-k|) = e^{k-q} (k<=q) + e^{q-k} (k>q).
    Per sample: single is_ge prefix masks (DVE/Pool) + indicator matmuls
    accumulate prefix tables A,C,PB,PD at G=128 grid points; totals TB,TD
    are extracted by a row-selector matmul and appended as constant table
    columns; one UNSCALED nearest-bin one-hot per query feeds one eval
    matmul producing all 6 rows; all 4 samples land in one [128,W] psum at
    32-aligned offsets.  Post-scaling by e^{-+q} ([128,W] exp with a
    per-partition sign column), a 0/+-1 combine matmul, reciprocal and
    multiply finish num/den -> h2.  e^k tables are scaled by 1/64 (cancels
    in num/den) for fp16 range safety.
  * DMA queueing matters on HW: broadcasts/loads on the two HWDGE queues
    (SP + ACT), never gpsimd/SWDGE; weight fp16 preloads are spread and
    overlapped; attention constants load during the AllToAll.
"""

import os
import sys

import numpy as np

for _p in ("/opt/trn_rl_repo", "/root/.axon_site/_ro/trn_rl_repo"):
    if os.path.isdir(_p) and _p not in sys.path:
        sys.path.insert(0, _p)

import concourse.bass as bass
import concourse.tile as tile
from concourse import bacc, mybir
from concourse.bass_utils import run_bass_kernel_spmd

F32 = mybir.dt.float32
F16 = mybir.dt.float16
ALU = mybir.AluOpType
ACTF = mybir.ActivationFunctionType

B = 32            # batch
W = 2048          # width (positions / features)
NCORES = 8
PCH = W // 128    # 16 partition chunks of the feature dim
FSL = W // NCORES  # 256 feature-slice per core
QKVW = 3 * FSL    # 768
SPC = B // NCORES  # 4 samples per core

G = 128           # grid bins for binned mode
LO, HI = -8.0, 8.0
DELTA = (HI - LO) / (G - 1)
HALF = DELTA / 2.0
EPS = 1e-6

MODE = os.environ.get("ATTN_MODE", "v2")
GROUPS = [list(range(NCORES))]


def _ap(tensor_handle, offset, ap):
    return bass.AP(tensor=tensor_handle, offset=offset, ap=ap)


def build(mode=None, reps=1, skip_gb=False, fake_cc=False,
          ohm_eng="dve", oh_bufs=2, mm16="dve", cc16=True, abl="full"):
    mode = mode or MODE
    nc = bacc.Bacc("TRN2", target_bir_lowering=False, debug=False,
                   num_devices=NCORES)

    x_t = nc.dram_tensor("x", [B, W], F32, kind="ExternalInput")
    gamma_t = nc.dram_tensor("gamma", [W], F32, kind="ExternalInput")
    beta_t = nc.dram_tensor("beta", [W], F32, kind="ExternalInput")
    wqkv_t = nc.dram_tensor("wqkv", [W, QKVW], F32, kind="ExternalInput")
    bqkv_t = nc.dram_tensor("bqkv", [QKVW], F32, kind="ExternalInput")
    wp_t = nc.dram_tensor("wp", [W, FSL], F32, kind="ExternalInput")
    bp_t = nc.dram_tensor("bp", [FSL], F32, kind="ExternalInput")
    xs_t = nc.dram_tensor("xs", [B, FSL], F32, kind="ExternalInput")
    out_t = nc.dram_tensor("out", [B, FSL], F32, kind="ExternalOutput")

    ccdt = F16 if cc16 else F32
    qkv_loc = nc.dram_tensor("qkv_loc", [B, QKVW], ccdt)
    qkv_a2a = nc.dram_tensor("qkv_a2a", [B, QKVW], ccdt)
    h2_loc = nc.dram_tensor("h2_loc", [SPC, W], ccdt)
    h2_gat = nc.dram_tensor("h2_gat", [B, W], ccdt, addr_space="Shared")

    c_eye32 = nc.inline_tensor(np.eye(32, dtype=np.float32), "c_eye32")
    c_eye8 = nc.inline_tensor(np.eye(8, dtype=np.float16), "c_eye8")
    c_eye8f = nc.inline_tensor(np.eye(8, dtype=np.float32), "c_eye8f")
    c_eye2 = nc.inline_tensor(np.eye(2, dtype=np.float32), "c_eye2")
    c_eye32_16 = nc.inline_tensor(np.eye(32, dtype=np.float16), "c_eye32_16")
    c_ones132 = nc.inline_tensor(np.ones((1, 32), np.float32), "c_ones132")
    gridv = np.linspace(LO, HI, G, dtype=np.float64).astype(np.float32)
    c_gcol = nc.inline_tensor(gridv.reshape(G, 1), "c_gcol")
    c_gcoln = nc.inline_tensor(-gridv.reshape(G, 1), "c_gcoln")
    c_grow = nc.inline_tensor(gridv.reshape(1, G), "c_grow")

    aps = dict(
        x=x_t.ap(), gamma=gamma_t.ap(), beta=beta_t.ap(),
        wqkv=wqkv_t.ap(), bqkv=bqkv_t.ap(), wp=wp_t.ap(), bp=bp_t.ap(),
        xs=xs_t.ap(), out=out_t.ap(),
        qkv_loc=qkv_loc.ap(), qkv_a2a=qkv_a2a.ap(),
        h2_loc=h2_loc.ap(), h2_gat=h2_gat.ap(),
        eye32=c_eye32.ap(), eye32_16=c_eye32_16.ap(),
        eye8=c_eye8.ap(), eye8f32=c_eye8f.ap(), eye2=c_eye2.ap(),
        ones132=c_ones132.ap(), gcol=c_gcol.ap(), gcoln=c_gcoln.ap(),
        grow=c_grow.ap(),
        a2a_tensor=qkv_a2a,
    )

    aps["fake_cc"] = fake_cc
    aps["ohm_eng"] = ohm_eng
    aps["oh_bufs"] = oh_bufs
    aps["mm16"] = mm16
    aps["cc16"] = cc16
    aps["abl"] = abl
    with tile.TileContext(nc) as tc:
        for _rep in range(reps):
            _build_tile(tc, aps, mode, skip_gb)

    nc.compile()
    return nc


def _build_tile(tc, aps, mode, skip_gb=False):
    nc = tc.nc

    with tc.tile_pool(name="singles", bufs=1) as singles:
        # ---- constants into SBUF ----
        eye32 = singles.tile([32, 32], F32)
        nc.sync.dma_start(eye32[:], aps["eye32"])
        eye32_16 = singles.tile([32, 32], F16)
        nc.sync.dma_start(eye32_16[:], aps["eye32_16"])
        eye8 = singles.tile([8, 8], F16 if aps["cc16"] else F32)
        nc.sync.dma_start(eye8[:], aps["eye8"]
                          if aps["cc16"] else aps["eye8f32"])
        eye2 = singles.tile([2, 2], F32)
        nc.sync.dma_start(eye2[:], aps["eye2"])
        ones132 = singles.tile([1, 32], F32)
        nc.sync.dma_start(ones132[:], aps["ones132"])
        gcol = singles.tile([G, 1], F32)
        nc.sync.dma_start(gcol[:], aps["gcol"])
        gcoln = singles.tile([G, 1], F32)
        nc.sync.dma_start(gcoln[:], aps["gcoln"])
        gbc = singles.tile([128, G], F32)
        nc.gpsimd.dma_start(gbc[:], aps["grow"].partition_broadcast(128))

        # ---- small weight bits ----
        bq32 = singles.tile([1, QKVW], F32)
        nc.sync.dma_start(bq32[:], aps["bqkv"].partition_broadcast(1))

        # residual + bp, exact fp32: xb = x_slice + bp
        xb = singles.tile([B, FSL], F32)
        bpb = singles.tile([B, FSL], F32)
        nc.gpsimd.dma_start(bpb[:], aps["bp"].partition_broadcast(B))
        xsl = singles.tile([B, FSL], F32)
        nc.sync.dma_start(xsl[:], aps["xs"])
        nc.vector.tensor_add(xb[:], xsl[:], bpb[:])

        # ---- layernorm (replicated, all 32 samples) ----
        sbx = singles.tile([B, W], F32, tag="bigio")
        nc.sync.dma_start(sbx[:], aps["x"])
        xg = sbx[:].rearrange("b (s f) -> b s f", s=4)  # 4 subgroups of 512
        stats = singles.tile([B, 4, 6], F32)
        for sg in range(4):
            nc.vector.bn_stats(stats[:, sg, :], xg[:, sg, :])
        mv = singles.tile([B, 2], F32)
        nc.vector.bn_aggr(mv[:], stats[:])
        eps_t = singles.tile([B, 1], F32)
        nc.vector.memset(eps_t[:], EPS)
        stdv = singles.tile([B, 1], F32)
        nc.scalar.activation(stdv[:], mv[:, 1:2], ACTF.Sqrt, bias=eps_t[:])
        rstd = singles.tile([B, 1], F32)
        nc.vector.reciprocal(rstd[:], stdv[:])
        h = singles.tile([B, W], F32)
        nc.vector.tensor_scalar(h[:], sbx[:], mv[:, 0:1], rstd[:],
                                op0=ALU.subtract, op1=ALU.mult)
        if not skip_gb:
            gb = singles.tile([B, W], F32, tag="gbb")
            nc.gpsimd.dma_start(gb[:], aps["gamma"].partition_broadcast(B))
            nc.vector.tensor_mul(h[:], h[:], gb[:])
            bb = singles.tile([B, W], F32, tag="gbb")
            nc.gpsimd.dma_start(bb[:], aps["beta"].partition_broadcast(B))
            nc.vector.tensor_add(h[:], h[:], bb[:])

        # ---- transpose h -> hT [128, PCH, 32] ----
        mm16 = aps["mm16"]
        wdt = F16 if mm16 != "off" else F32
        hT = singles.tile([128, PCH, B], wdt)
        with tc.tile_pool(name="ptr", bufs=2, space="PSUM") as ptr_pool:
            for ci in range(PCH):
                ptr = ptr_pool.tile([128, B], F32)
                nc.tensor.transpose(ptr[:], h[:, ci * 128:(ci + 1) * 128],
                                    eye32[:])
                nc.vector.tensor_copy(hT[:, ci, :], ptr[:])

        # ---- qkv matmul: [32, 768] = h @ wqkv + bqkv ----
        sbq = singles.tile([B, QKVW], F16 if aps["cc16"] else F32)
        with (
            tc.tile_pool(name="pq", bufs=1, space="PSUM") as pq_pool,
            tc.tile_pool(name="wst", bufs=4) as wst_pool,
        ):
            pq = pq_pool.tile([B, QKVW], F32)
            for ci in range(PCH):
                wch = wst_pool.tile([128, QKVW], F32, tag="wch")
                nc.sync.dma_start(wch[:],
                                  aps["wqkv"][ci * 128:(ci + 1) * 128, :])
                if mm16 == "off":
                    wmm = wch
                else:
                    wmm = wst_pool.tile([128, QKVW], F16, tag="wch16")
                    nc.vector.tensor_copy(wmm[:], wch[:])
                nc.tensor.matmul(pq[:, 0:512], hT[:, ci, :],
                                 wmm[:, 0:512],
                                 start=(ci == 0), stop=False)
                nc.tensor.matmul(pq[:, 512:QKVW], hT[:, ci, :],
                                 wmm[:, 512:QKVW],
                                 start=(ci == 0), stop=False)
            nc.tensor.matmul(pq[:, 0:512], ones132[:], bq32[:, 0:512],
                             start=False, stop=True)
            nc.tensor.matmul(pq[:, 512:QKVW], ones132[:], bq32[:, 512:QKVW],
                             start=False, stop=True)
            nc.vector.tensor_copy(sbq[:], pq[:])
        nc.sync.dma_start(aps["qkv_loc"], sbq[:])

        if aps.get("fake_cc"):
            nc.sync.dma_start(aps["qkv_a2a"], aps["qkv_loc"])
        else:
            nc.gpsimd.collective_compute(
                "AllToAll", ALU.bypass, replica_groups=GROUPS,
                ins=[aps["qkv_loc"]], outs=[aps["qkv_a2a"]])

        # ---- attention (4 samples) ----
        abl = aps["abl"]
        num_t = singles.tile([SPC, W], F32)
        den_t = singles.tile([SPC, W], F32)
        shared = dict(a2a=aps["a2a_tensor"], num=num_t, den=den_t,
                      eye8=eye8, eye2=eye2, gbc=gbc, gcol=gcol,
                      gcoln=gcoln, ohm_eng=aps["ohm_eng"],
                      oh_bufs=aps["oh_bufs"],
                      ccdt=F16 if aps["cc16"] else F32)
        if abl in ("no_attn", "qkv_only"):
            nc.vector.memset(num_t[:], 1.0)
            nc.vector.memset(den_t[:], 1.0)
        elif mode == "binned":
            _attn_binned(tc, shared)
        else:
            _attn_naive(tc, shared)

        dinv = singles.tile([SPC, W], F32)
        nc.vector.reciprocal(dinv[:], den_t[:])
        sbh2 = singles.tile([SPC, W], F16 if aps["cc16"] else F32)
        nc.vector.tensor_mul(sbh2[:], num_t[:], dinv[:])
        nc.sync.dma_start(aps["h2_loc"], sbh2[:])

        if abl in ("no_proj", "qkv_only"):
            nc.sync.dma_start(aps["out"], xb[:])
            return
        if aps.get("fake_cc"):
            nc.sync.dma_start(aps["h2_gat"][0:SPC, :], aps["h2_loc"])
        else:
            nc.gpsimd.collective_compute(
                "AllGather", ALU.bypass, replica_groups=GROUPS,
                ins=[aps["h2_loc"]], outs=[aps["h2_gat"]])

        # ---- output projection ----
        h2dt = F16 if aps["cc16"] else F32
        h2f = singles.tile([B, W], h2dt, tag="bigio2")
        nc.sync.dma_start(h2f[:], aps["h2_gat"])
        h2T = singles.tile([128, PCH, B], wdt)
        eyeh2 = eye32_16 if aps["cc16"] else eye32
        with tc.tile_pool(name="ptr2", bufs=2, space="PSUM") as ptr2_pool:
            for ci in range(PCH):
                ptr2 = ptr2_pool.tile([128, B], h2dt)
                nc.tensor.transpose(ptr2[:], h2f[:, ci * 128:(ci + 1) * 128],
                                    eyeh2[:])
                nc.vector.tensor_copy(h2T[:, ci, :], ptr2[:])

        sbo = singles.tile([B, FSL], F32)
        with (
            tc.tile_pool(name="pout", bufs=1, space="PSUM") as pout_pool,
            tc.tile_pool(name="wpst", bufs=4) as wpst_pool,
        ):
            pout = pout_pool.tile([B, FSL], F32)
            for ci in range(PCH):
                wpch = wpst_pool.tile([128, FSL], F32, tag="wpch")
                nc.sync.dma_start(wpch[:],
                                  aps["wp"][ci * 128:(ci + 1) * 128, :])
                if mm16 == "off":
                    wpmm = wpch
                else:
                    wpmm = wpst_pool.tile([128, FSL], F16, tag="wpch16")
                    nc.vector.tensor_copy(wpmm[:], wpch[:])
                nc.tensor.matmul(pout[:], h2T[:, ci, :], wpmm[:],
                                 start=(ci == 0), stop=(ci == PCH - 1))
            nc.vector.tensor_add(sbo[:], pout[:], xb[:])
        nc.sync.dma_start(aps["out"], sbo[:])


def _load_qkv_sample(nc, kv_pool, ptp_pool, shared, s):
    """Per-sample loads from the AllToAll result: broadcast q [128, W] and
    k/v transposed into [128, 16] (feature chunk m = half*8 + coreblk)."""
    a2a = shared["a2a"]
    eye8 = shared["eye8"]
    cdt = shared["ccdt"]
    dma = nc.sync.dma_start if cdt == F16 else nc.gpsimd.dma_start
    row_k = kv_pool.tile([8, 256], cdt, tag="krow")
    dma(row_k[:], _ap(a2a, s * QKVW + FSL, [[4 * QKVW, 8], [1, 256]]))
    row_v = kv_pool.tile([8, 256], cdt, tag="vrow")
    dma(row_v[:], _ap(a2a, s * QKVW + 2 * FSL, [[4 * QKVW, 8], [1, 256]]))
    kTt = kv_pool.tile([128, PCH], F32, tag="kT")
    vTt = kv_pool.tile([128, PCH], F32, tag="vT")
    for half in range(2):
        ptk = ptp_pool.tile([128, 8], cdt, tag="ptp")
        nc.tensor.transpose(ptk[:], row_k[:, half * 128:(half + 1) * 128],
                            eye8[:])
        nc.vector.tensor_copy(kTt[:, half * 8:(half + 1) * 8], ptk[:])
        ptv = ptp_pool.tile([128, 8], cdt, tag="ptp")
        nc.tensor.transpose(ptv[:], row_v[:, half * 128:(half + 1) * 128],
                            eye8[:])
        nc.vector.tensor_copy(vTt[:, half * 8:(half + 1) * 8], ptv[:])
    return kTt, vTt


def _q_broadcast(nc, pool, shared, s, clamp):
    qb = pool.tile([128, W], shared["ccdt"], tag="qb")
    src = _ap(shared["a2a"], s * QKVW, [[0, 128], [4 * QKVW, 8], [1, 256]])
    if shared["ccdt"] == F16:
        nc.sync.dma_start(qb[:], src)
    else:
        nc.gpsimd.dma_start(qb[:], src)
    if clamp:
        nc.vector.tensor_scalar(qb[:], qb[:], LO, HI,
                                op0=ALU.max, op1=ALU.min)
    return qb


def _attn_binned(tc, shared):
    nc = tc.nc
    gbc = shared["gbc"]
    gcoln = shared["gcoln"]
    eye2 = shared["eye2"]
    ohm_op = (nc.gpsimd.tensor_mul if shared["ohm_eng"] == "gpsimd"
              else nc.vector.tensor_mul)
    with (
        tc.tile_pool(name="akv", bufs=2) as kv_pool,
        tc.tile_pool(name="aqb", bufs=2) as qb_pool,
        tc.tile_pool(name="aoh", bufs=shared["oh_bufs"]) as oh_pool,
        tc.tile_pool(name="amk", bufs=3) as mk_pool,
        tc.tile_pool(name="atab", bufs=2) as tab_pool,
        tc.tile_pool(name="ptp", bufs=2, space="PSUM") as ptp_pool,
        tc.tile_pool(name="ptab", bufs=2, space="PSUM") as ptab_pool,
        tc.tile_pool(name="pnd", bufs=1, space="PSUM") as pnd_pool,
    ):
        for s in range(SPC):
            qb = _q_broadcast(nc, qb_pool, shared, s, clamp=False)
            kTt, vTt = _load_qkv_sample(nc, kv_pool, ptp_pool, shared, s)

            ek = kv_pool.tile([128, PCH], F32, tag="ek")
            nc.scalar.activation(ek[:], kTt[:], ACTF.Exp)
            emk = kv_pool.tile([128, PCH], F32, tag="emk")
            nc.scalar.activation(emk[:], kTt[:], ACTF.Exp, scale=-1.0)
            u = kv_pool.tile([128, PCH, 4], F16, tag="u")
            nc.vector.tensor_mul(u[:, :, 0], ek[:], vTt[:])
            nc.vector.tensor_copy(u[:, :, 1], ek[:])
            nc.vector.tensor_mul(u[:, :, 2], emk[:], vTt[:])
            nc.vector.tensor_copy(u[:, :, 3], emk[:])

            # cumulative tables at the G grid points: psum rows = u-type
            ptab = ptab_pool.tile([4, 2 * G], F32, tag="ptab")
            for m in range(PCH):
                mk = mk_pool.tile([128, 2 * G], F16, tag="mk")
                nc.vector.tensor_scalar(mk[:, 0:G], gbc[:],
                                        kTt[:, m:m + 1], None, op0=ALU.is_ge)
                nc.vector.tensor_scalar(mk[:, G:2 * G], gbc[:],
                                        kTt[:, m:m + 1], None, op0=ALU.is_lt)
                nc.tensor.matmul(ptab[:], u[:, m, :], mk[:],
                                 start=(m == 0), stop=(m == PCH - 1))
            # rows 0,1 x cols [0,G)  = A,C (prefix with e^k);
            # rows 2,3 x cols [G,2G) = B,D (suffix with e^-k)
            sbtab = tab_pool.tile([4, 2 * G], F32, tag="sbtab")
            nc.scalar.copy(sbtab[:], ptab[:])
            sbBD = tab_pool.tile([2, G], F32, tag="sbBD")
            nc.sync.dma_start(sbBD[:], sbtab[2:4, G:2 * G])
            tabs = tab_pool.tile([G, 4], F16, tag="tabs")
            ptt = ptp_pool.tile([G, 2], F32, tag="ptp")
            nc.tensor.transpose(ptt[:], sbtab[0:2, 0:G], eye2[:])
            nc.vector.tensor_copy(tabs[:, 0:2], ptt[:])
            ptt2 = ptp_pool.tile([G, 2], F32, tag="ptp")
            nc.tensor.transpose(ptt2[:], sbBD[:], eye2[:])
            nc.vector.tensor_copy(tabs[:, 2:4], ptt2[:])

            # one-hot of nearest grid point, pre-scaled by e^{-+q}
            t1 = qb_pool.tile([128, W], F32, tag="t1", bufs=2)
            nc.scalar.activation(t1[:], qb[:], ACTF.Abs, bias=gcoln[:])
            oh = oh_pool.tile([128, W], F16, tag="oh")
            nc.vector.tensor_scalar(oh[:], t1[:], HALF, None, op0=ALU.is_le)
            emq = oh_pool.tile([128, W], F16, tag="emq")
            nc.scalar.activation(emq[:], qb[:], ACTF.Exp, scale=-1.0)
            epq = oh_pool.tile([128, W], F16, tag="epq")
            nc.scalar.activation(epq[:], qb[:], ACTF.Exp, scale=1.0)
            ohm = oh_pool.tile([128, W], F16, tag="ohm")
            ohm_op(ohm[:], oh[:], emq[:])
            ohp = oh_pool.tile([128, W], F16, tag="ohp")
            ohm_op(ohp[:], oh[:], epq[:])

            pnd = pnd_pool.tile([2, W], F32, tag="pnd")
            for n in range(4):
                sl = slice(n * 512, (n + 1) * 512)
                nc.tensor.matmul(pnd[:, sl], tabs[:, 0:2], ohm[:, sl],
                                 start=True, stop=False)
                nc.tensor.matmul(pnd[:, sl], tabs[:, 2:4], ohp[:, sl],
                                 start=False, stop=True)
            ns_s = oh_pool.tile([2, W], F32, tag="ns")
            nc.scalar.copy(ns_s[:], pnd[:])
            nc.sync.dma_start(shared["num"][s:s + 1, :], ns_s[0:1, :])
            nc.sync.dma_start(shared["den"][s:s + 1, :], ns_s[1:2, :])


def _attn_naive(tc, shared):
    nc = tc.nc
    with (
        tc.tile_pool(name="akv", bufs=2) as kv_pool,
        tc.tile_pool(name="aqb", bufs=2) as qb_pool,
        tc.tile_pool(name="aab", bufs=2) as ab_pool,
        tc.tile_pool(name="apt", bufs=3) as pt_pool,
        tc.tile_pool(name="ptp", bufs=2, space="PSUM") as ptp_pool,
        tc.tile_pool(name="pnd", bufs=1, space="PSUM") as pnd_pool,
    ):
        for s in range(SPC):
            qb = _q_broadcast(nc, qb_pool, shared, s, clamp=False)
            kTt, vTt = _load_qkv_sample(nc, kv_pool, ptp_pool, shared, s)

            nk = kv_pool.tile([128, PCH], F32, tag="nk")
            nc.vector.tensor_scalar(nk[:], kTt[:], -1.0, None, op0=ALU.mult)
            u2 = kv_pool.tile([128, PCH, 2], F16, tag="u2")
            nc.vector.tensor_copy(u2[:, :, 0], vTt[:])
            nc.vector.memset(u2[:, :, 1], 1.0)

            pnd = pnd_pool.tile([2, W], F32, tag="pnd")
            for m in range(PCH):
                ab = ab_pool.tile([128, W], F32, tag="ab")
                nc.scalar.activation(ab[:], qb[:], ACTF.Abs,
                                     bias=nk[:, m:m + 1])
                pt = pt_pool.tile([128, W], F16, tag="pt")
                nc.scalar.activation(pt[:], ab[:], ACTF.Exp, scale=-1.0)
                for n in range(4):
                    sl = slice(n * 512, (n + 1) * 512)
                    nc.tensor.matmul(pnd[:, sl], u2[:, m, :], pt[:, sl],
                                     start=(m == 0), stop=(m == PCH - 1))
            ns_s = ab_pool.tile([2, W], F32, tag="ns")
            nc.scalar.copy(ns_s[:], pnd[:])
            nc.sync.dma_start(shared["num"][s:s + 1, :], ns_s[0:1, :])
            nc.sync.dma_start(shared["den"][s:s + 1, :], ns_s[1:2, :])


# ---------------------------------------------------------------------------
# v2: restructured kernel.
#   LN stats via matmul on [128,512] layout; fp16 weights (host-cast);
#   binned attention with prefix-only masks, shared unscaled one-hot,
#   batched post-scaling, all num/den in one PSUM tile; feature-par proj.
# ---------------------------------------------------------------------------

LN64 = float(np.log(64.0))


def _v2_consts(nc):
    G_ = G
    gridv = np.linspace(LO, HI, G_, dtype=np.float64).astype(np.float32)
    c = {}
    c["eye8"] = nc.inline_tensor(np.eye(8, dtype=np.float16), "v2_eye8")
    c["eye4"] = nc.inline_tensor(np.eye(4, dtype=np.float16), "v2_eye4")
    rs = np.zeros((128, G_), np.float16)
    rs[G_ - 1, :] = 1.0
    c["rowsel"] = nc.inline_tensor(rs, "v2_rowsel")
    c["eye32"] = nc.inline_tensor(np.eye(32, dtype=np.float16), "v2_eye32")
    c["eye128"] = nc.inline_tensor(np.eye(128, dtype=np.float16), "v2_eye128")
    c["ones1"] = nc.inline_tensor(np.ones((1, 32), np.float16), "v2_ones1")
    c["gcoln"] = nc.inline_tensor(-gridv.reshape(G_, 1), "v2_gcoln")
    c["gcol"] = nc.inline_tensor(gridv.reshape(G_, 1), "v2_gcol")
    c["gbc16"] = nc.inline_tensor(
        np.tile(gridv.reshape(1, G_), (128, 1)).astype(np.float16), "v2_gbc16")
    # stats: selT[p, s] = 1 if p//4 == s  (x128 partition p = s*4 + c)
    selT = np.zeros((128, 32), np.float16)
    for p in range(128):
        selT[p, p // 4] = 1.0
    c["selT"] = nc.inline_tensor(selT, "v2_selT")
    # sign column for E = exp(sgn * q): rows (A,C -> -1), (B,D -> +1)
    sgn = np.ones((128, 1), np.float32)
    for s in range(4):
        sgn[32 * s + 0] = -1.0
        sgn[32 * s + 1] = -1.0
    c["sgn"] = nc.inline_tensor(sgn, "v2_sgn")
    # num_s = emq*A - epq*PB + epq*TB ; den_s = emq*C - epq*PD + epq*TD
    cmb = np.zeros((128, 36), np.float16)
    for s in range(4):
        cmb[32 * s + 0, s] = 1.0
        cmb[32 * s + 2, s] = -1.0
        cmb[32 * s + 4, s] = 1.0
        cmb[32 * s + 1, 32 + s] = 1.0
        cmb[32 * s + 3, 32 + s] = -1.0
        cmb[32 * s + 5, 32 + s] = 1.0
    c["cmb"] = nc.inline_tensor(cmb, "v2_cmb")
    return c


def build_v2(reps=1, opts=None):
    nc = bacc.Bacc("TRN2", target_bir_lowering=False, debug=False,
                   num_devices=NCORES)

    x_t = nc.dram_tensor("x", [B, W], F32, kind="ExternalInput")
    wqkv_t = nc.dram_tensor("wqkv", [W, QKVW], F16, kind="ExternalInput")
    wqcs_t = nc.dram_tensor("wqcs", [1, QKVW], F16, kind="ExternalInput")
    bqkv_t = nc.dram_tensor("bqkv", [1, QKVW], F16, kind="ExternalInput")
    wp_t = nc.dram_tensor("wp", [W, FSL], F16, kind="ExternalInput")
    bp_t = nc.dram_tensor("bp", [FSL], F32, kind="ExternalInput")
    xs_t = nc.dram_tensor("xs", [B, FSL], F32, kind="ExternalInput")
    out_t = nc.dram_tensor("out", [B, FSL], F32, kind="ExternalOutput")

    qkv_loc = nc.dram_tensor("qkv_loc", [B, QKVW], F16)
    qkv_a2a = nc.dram_tensor("qkv_a2a", [B, QKVW], F16)
    h2_loc = nc.dram_tensor("h2_loc", [SPC, W], F16)
    h2_gat = nc.dram_tensor("h2_gat", [B, W], F16, addr_space="Shared")
    h2t_loc = nc.dram_tensor("h2t_loc", [128, PCH * SPC], F16)
    h2t_gat = nc.dram_tensor("h2t_gat", [128 * NCORES, PCH * SPC], F16,
                             addr_space="Shared")

    consts = _v2_consts(nc)

    aps = dict(
        x=x_t, wqkv=wqkv_t, wqcs=wqcs_t, bqkv=bqkv_t, wp=wp_t, bp=bp_t,
        xs=xs_t,
        out=out_t, qkv_loc=qkv_loc, qkv_a2a=qkv_a2a,
        h2_loc=h2_loc, h2_gat=h2_gat, h2t_loc=h2t_loc, h2t_gat=h2t_gat,
        consts=consts,
    )
    aps["opts"] = dict(qb_eng="sync", qq_eng="sync", kv_eng="scalar",
                       masks="prebuild", tail="halves",
                       hT="pe", h2T="pe", stats_sq="pool", newton=True,
                       wdma="mix", qq_late=False, warm=0,
                       t1_eng="act", mask_dve_mod=4, ag_t=False)
    aps["opts"].update(opts or {})
    with tile.TileContext(nc) as tc:
        for _rep in range(reps):
            _build_tile_v2(tc, aps)
    nc.compile()
    return nc


def _build_tile_v2(tc, aps):
    nc = tc.nc
    C = aps["consts"]
    O = aps["opts"]

    with tc.tile_pool(name="v2s", bufs=1) as sg:
        # ---- input DMAs first (critical path) ----
        x128 = sg.tile([128, 512], F32)
        nc.sync.dma_start(x128[:], _ap(aps["x"], 0, [[512, 128], [1, 512]]))
        selT = sg.tile([128, 32], F16)
        nc.scalar.dma_start(selT[:], C["selT"].ap())
        eye128 = sg.tile([128, 128], F16)
        nc.scalar.dma_start(eye128[:], C["eye128"].ap())
        xsl = sg.tile([B, FSL], F32)
        nc.sync.dma_start(xsl[:], aps["xs"].ap())
        # LN inputs squared + f16 copy, ahead of any weight traffic
        xf = sg.tile([128, 1024], F16)
        nc.vector.tensor_copy(xf[:, 0:512], x128[:])
        if O["stats_sq"] == "pool":
            nc.gpsimd.tensor_mul(xf[:, 512:1024], x128[:], x128[:])
        else:
            nc.scalar.activation(xf[:, 512:1024], x128[:], ACTF.Square)
        wcb = sg.tile([2, QKVW], F16)
        nc.sync.dma_start(wcb[0:1, :], aps["wqcs"].ap())
        nc.sync.dma_start(wcb[1:2, :], aps["bqkv"].ap())
        eye32 = sg.tile([32, 32], F16)
        nc.scalar.dma_start(eye32[:], C["eye32"].ap())
        ones1 = sg.tile([1, 32], F16)
        nc.sync.dma_start(ones1[:], C["ones1"].ap())
        bq16 = sg.tile([1, QKVW], F16)
        nc.scalar.dma_start(bq16[:], aps["bqkv"].ap())
        mln64 = sg.tile([128, 1], F32)
        nc.vector.memset(mln64[:], -LN64)

        # ---- weights (fp16, preloaded; spread across queues) ----
        if O["wdma"] == "hwdge":
            wengs = (nc.scalar, nc.sync)
        else:
            wengs = (nc.scalar, nc.gpsimd)
        wst = sg.tile([128, PCH, QKVW], F16, tag="wst")
        for i in range(8):
            src = _ap(aps["wqkv"], i * 2 * 128 * QKVW,
                      [[QKVW, 128], [128 * QKVW, 2], [1, QKVW]])
            wengs[i % 2].dma_start(wst[:, 2 * i:2 * i + 2, :], src)
        wpst = sg.tile([128, PCH, FSL], F16, tag="wpst")
        for i in range(4):
            src = _ap(aps["wp"], i * 4 * 128 * FSL,
                      [[FSL, 128], [128 * FSL, 4], [1, FSL]])
            wengs[i % 2].dma_start(wpst[:, 4 * i:4 * i + 4, :], src)

        # residual + bp: xb = x_slice + bp
        xb = sg.tile([B, FSL], F32)
        bpb = sg.tile([B, FSL], F32)
        beng = nc.scalar if O["wdma"] == "hwdge" else nc.gpsimd
        beng.dma_start(bpb[:], aps["bp"].ap().partition_broadcast(B))
        nc.vector.tensor_add(xb[:], xsl[:], bpb[:])

        # ---- layernorm stats on [128, 512] layout ----
        with tc.tile_pool(name="v2pst", bufs=1, space="PSUM") as pst_pool:
            pstat = pst_pool.tile([32, 1024], F32)
            nc.tensor.matmul(pstat[:, 0:512], selT[:], xf[:, 0:512],
                             start=True, stop=True)
            nc.tensor.matmul(pstat[:, 512:1024], selT[:], xf[:, 512:1024],
                             start=True, stop=True)
            reds = sg.tile([32, 2], F32)
            nc.vector.tensor_reduce(reds[:, 0:1], pstat[:, 0:512],
                                    axis=mybir.AxisListType.X,
                                    op=ALU.add)
            nc.vector.tensor_reduce(reds[:, 1:2], pstat[:, 512:1024],
                                    axis=mybir.AxisListType.X,
                                    op=ALU.add)
        m2 = sg.tile([32, 2], F32)
        nc.vector.tensor_scalar(m2[:], reds[:], 1.0 / W, None, op0=ALU.mult)
        musq = sg.tile([32, 1], F32)
        nc.vector.tensor_mul(musq[:], m2[:, 0:1], m2[:, 0:1])
        var = sg.tile([32, 1], F32)
        nc.vector.tensor_sub(var[:], m2[:, 1:2], musq[:])
        # rstd via Newton rsqrt (var ~ 1, 3 iterations, all on DVE)
        y1 = sg.tile([32, 1], F32)
        nc.vector.tensor_scalar(y1[:], var[:], -0.5, 1.5,
                                op0=ALU.mult, op1=ALU.add)
        t_n = sg.tile([32, 4], F32, tag="newt")
        nc.vector.tensor_mul(t_n[:, 0:1], y1[:], y1[:])
        nc.vector.tensor_mul(t_n[:, 1:2], t_n[:, 0:1], var[:])
        nc.vector.tensor_scalar(t_n[:, 2:3], t_n[:, 1:2], -0.5, 1.5,
                                op0=ALU.mult, op1=ALU.add)
        y2 = sg.tile([32, 1], F32)
        nc.vector.tensor_mul(y2[:], y1[:], t_n[:, 2:3])
        t_m = sg.tile([32, 4], F32, tag="newt")
        nc.vector.tensor_mul(t_m[:, 0:1], y2[:], y2[:])
        nc.vector.tensor_mul(t_m[:, 1:2], t_m[:, 0:1], var[:])
        nc.vector.tensor_scalar(t_m[:, 2:3], t_m[:, 1:2], -0.5, 1.5,
                                op0=ALU.mult, op1=ALU.add)
        rstd32 = sg.tile([32, 1], F32)
        nc.vector.tensor_mul(rstd32[:], y2[:], t_m[:, 2:3])
        # mu_std cols (mu, std);  std = var * rstd = sqrt(var)
        mu_std = sg.tile([32, 2], F16)
        nc.vector.tensor_copy(mu_std[:, 0:1], m2[:, 0:1])
        nc.vector.tensor_mul(mu_std[:, 1:2], var[:], rstd32[:])
        musrow = sg.tile([2, 32], F16)
        with tc.tile_pool(name="v2pmu", bufs=1, space="PSUM") as pmu_pool:
            pmu = pmu_pool.tile([2, 32], F32)
            nc.tensor.matmul(pmu[:], mu_std[:], eye32[:],
                             start=True, stop=True)
            nc.vector.tensor_copy(musrow[:], pmu[:])

        # ---- transpose raw x16 -> xT [128, PCH, 32] (f16) ----
        hT = sg.tile([128, PCH, B], F16)
        hTv = hT[:].rearrange("p (c b) s -> p b c s", b=4)
        with tc.tile_pool(name="v2ptr", bufs=2, space="PSUM") as ptr_pool:
            for bb in range(4):
                ptr = ptr_pool.tile([128, 128], F16)
                nc.tensor.transpose(ptr[:],
                                    xf[:, bb * 128:(bb + 1) * 128],
                                    eye128[:])
                ptr_r = ptr[:].rearrange("p (s c) -> p c s", c=4)
                nc.vector.tensor_copy(hTv[:, bb, :, :], ptr_r[:])

        # ---- qkv matmul on raw x; LN folded in afterwards ----
        # qkv = rstd*(xq + mu*(-S) + std*b)  with S = colsum(wqkv)
        sbq = sg.tile([B, QKVW], F16)
        with tc.tile_pool(name="v2pq", bufs=1, space="PSUM") as pq_pool:
            pq = pq_pool.tile([B, QKVW], F32)
            for m in range(PCH):
                nc.tensor.matmul(pq[:, 0:512], hT[:, m, :], wst[:, m, 0:512],
                                 start=(m == 0), stop=False)
                nc.tensor.matmul(pq[:, 512:QKVW], hT[:, m, :],
                                 wst[:, m, 512:QKVW],
                                 start=(m == 0), stop=False)
            for sl in (slice(0, 512), slice(512, QKVW)):
                nc.tensor.matmul(pq[:, sl], musrow[:], wcb[:, sl],
                                 start=False, stop=True)
            nc.vector.tensor_scalar(sbq[:], pq[:], rstd32[:], None,
                                    op0=ALU.mult)
        nc.sync.dma_start(aps["qkv_loc"].ap(), sbq[:])

        nc.gpsimd.collective_compute(
            "AllToAll", ALU.bypass, replica_groups=GROUPS,
            ins=[aps["qkv_loc"].ap()], outs=[aps["qkv_a2a"].ap()])

        # constants for the attention phase: load during the AllToAll
        eye4 = sg.tile([4, 4], F16)
        nc.sync.dma_start(eye4[:], C["eye4"].ap())
        eye8 = sg.tile([8, 8], F16)
        nc.sync.dma_start(eye8[:], C["eye8"].ap())
        rowsel = sg.tile([128, G], F16)
        nc.sync.dma_start(rowsel[:], C["rowsel"].ap())
        gcoln = sg.tile([G, 1], F32)
        nc.sync.dma_start(gcoln[:], C["gcoln"].ap())
        gcol = sg.tile([G, 1], F32)
        nc.sync.dma_start(gcol[:], C["gcol"].ap())
        gbc16 = sg.tile([128, G], F16)
        nc.sync.dma_start(gbc16[:], C["gbc16"].ap())
        sgn = sg.tile([128, 1], F32)
        nc.sync.dma_start(sgn[:], C["sgn"].ap())
        cmb = sg.tile([128, 36], F16)
        nc.sync.dma_start(cmb[:], C["cmb"].ap())

        # ---- attention (4 samples) ----
        a2a = aps["qkv_a2a"]
        QQ = sg.tile([128, W], F16, tag="QQ")
        h2sb = sg.tile([SPC, W], F16)
        numsb = sg.tile([SPC, W], F16)
        dinv = sg.tile([SPC, W], F32)
        with tc.tile_pool(name="v2pnd", bufs=1, space="PSUM") as pnd_pool:
            pnd = pnd_pool.tile([128, W], F32)
            with (
                tc.tile_pool(name="v2kv", bufs=2) as kv_pool,
                tc.tile_pool(name="v2qb", bufs=2) as qb_pool,
                tc.tile_pool(name="v2oh", bufs=2) as oh_pool,
                tc.tile_pool(name="v2mk", bufs=2) as mk_pool,
                tc.tile_pool(name="v2ptab", bufs=2, space="PSUM") as ptab_pool,
                tc.tile_pool(name="v2ptp", bufs=2, space="PSUM") as ptp_pool,
            ):
                engs = {"scalar": nc.scalar, "sync": nc.sync,
                        "gpsimd": nc.gpsimd}

                def _qq_dma(s):
                    engs[O["qq_eng"]].dma_start(
                        QQ[32 * s:32 * s + 32, :],
                        _ap(a2a, s * QKVW,
                            [[0, 32], [4 * QKVW, 8], [1, 256]]))

                for s in range(SPC):
                    # q rows for the post-scale (broadcast to whole block)
                    if not O["qq_late"]:
                        _qq_dma(s)
                    # k,v transposed loads straight from DRAM (strided)
                    kT16 = kv_pool.tile([128, PCH], F16, tag="kT16")
                    vTt = kv_pool.tile([128, PCH], F16, tag="vT")
                    kv_row = kv_pool.tile([8, 512], F16, tag="kvrow")
                    engs[O["kv_eng"]].dma_start(
                        kv_row[:],
                        _ap(a2a, s * QKVW + FSL, [[4 * QKVW, 8], [1, 512]]))
                    for half in range(2):
                        ptk = ptp_pool.tile([128, 8], F16, tag="ptp")
                        nc.tensor.transpose(
                            ptk[:], kv_row[:, half * 128:(half + 1) * 128],
                            eye8[:])
                        nc.vector.tensor_copy(
                            kT16[:, half * 8:(half + 1) * 8], ptk[:])
                        ptv = ptp_pool.tile([128, 8], F16, tag="ptp")
                        nc.tensor.transpose(
                            ptv[:],
                            kv_row[:, 256 + half * 128:256 + (half + 1) * 128],
                            eye8[:])
                        nc.vector.tensor_copy(
                            vTt[:, half * 8:(half + 1) * 8], ptv[:])
                    kTt = kv_pool.tile([128, PCH], F32, tag="kT")
                    nc.vector.tensor_copy(kTt[:], kT16[:])

                    # u-vectors: e^k/64 * {v, 1}, e^-k/64 * {v, 1}
                    ek = kv_pool.tile([128, PCH], F16, tag="ek")
                    nc.scalar.activation(ek[:], kT16[:], ACTF.Exp,
                                         bias=mln64[:])
                    emk = kv_pool.tile([128, PCH], F16, tag="emk")
                    nc.scalar.activation(emk[:], kT16[:], ACTF.Exp,
                                         bias=mln64[:], scale=-1.0)
                    u = kv_pool.tile([128, PCH, 4], F16, tag="u")
                    nc.vector.tensor_mul(u[:, :, 0], ek[:], vTt[:])
                    nc.vector.tensor_copy(u[:, :, 1], ek[:])
                    nc.gpsimd.tensor_mul(u[:, :, 2], emk[:], vTt[:])
                    nc.gpsimd.tensor_copy(u[:, :, 3], emk[:])

                    # prefix masks for all chunks, then the table matmuls
                    ptab = ptab_pool.tile([4, G], F32, tag="ptab")
                    if O["masks"] == "prebuild":
                        mk_all = mk_pool.tile([128, PCH, G], F16, tag="mk")
                        for m in range(PCH):
                            eng = (nc.vector if (m % O["mask_dve_mod"] == 0)
                                   else nc.gpsimd)
                            eng.tensor_scalar(mk_all[:, m, :], gbc16[:],
                                              kTt[:, m:m + 1], None,
                                              op0=ALU.is_ge)
                        for m in range(PCH):
                            nc.tensor.matmul(ptab[:], u[:, m, :],
                                             mk_all[:, m, :],
                                             start=(m == 0),
                                             stop=(m == PCH - 1))
                    else:
                        for m in range(PCH):
                            mk = mk_pool.tile([128, G], F16, tag="mk")
                            eng = nc.vector if (m % 2 == 0) else nc.gpsimd
                            eng.tensor_scalar(mk[:], gbc16[:],
                                              kTt[:, m:m + 1], None,
                                              op0=ALU.is_ge)
                            nc.tensor.matmul(ptab[:], u[:, m, :], mk[:],
                                             start=(m == 0),
                                             stop=(m == PCH - 1))
                    sbtab = kv_pool.tile([4, G], F16, tag="sbtab")
                    nc.vector.tensor_copy(sbtab[:], ptab[:])
                    # tabs6 cols: A, C, PB, PD, TB, TD  (prefix tables +
                    # constant totals; suffix = TB - PB folded via cmb sign)
                    tabs6 = kv_pool.tile([G, 32], F16, tag="tabs")
                    nc.vector.memset(tabs6[:, 6:32], 0.0)
                    ptt = ptp_pool.tile([G, 4], F16, tag="ptp")
                    nc.tensor.transpose(ptt[:], sbtab[:], eye4[:])
                    nc.vector.tensor_copy(tabs6[:, 0:4], ptt[:])
                    ptt2 = ptp_pool.tile([G, 2], F32, tag="ptp")
                    nc.tensor.matmul(ptt2[:], rowsel[:], tabs6[:, 2:4],
                                     start=True, stop=True)
                    nc.vector.tensor_copy(tabs6[:, 4:6], ptt2[:])

                    # one-hot of nearest grid point (shared by all 4 rows)
                    qb = qb_pool.tile([128, W], F16, tag="qb")
                    engs[O["qb_eng"]].dma_start(
                        qb[:],
                        _ap(a2a, s * QKVW,
                            [[0, 128], [4 * QKVW, 8], [1, 256]]))
                    t1 = qb_pool.tile([128, W], F16, tag="t1")
                    if O["t1_eng"] == "dve":
                        nc.vector.tensor_scalar(t1[:], qb[:], gcol[:], 0.0,
                                                op0=ALU.subtract,
                                                op1=ALU.abs_max)
                    else:
                        nc.scalar.activation(t1[:], qb[:], ACTF.Abs,
                                             bias=gcoln[:])
                    oh = oh_pool.tile([128, W], F16, tag="oh")
                    nc.vector.tensor_scalar(oh[:], t1[:], HALF, None,
                                            op0=ALU.is_le)

                    for n in range(4):
                        sl = slice(n * 512, (n + 1) * 512)
                        nc.tensor.matmul(pnd[32 * s:32 * s + 32, sl],
                                         tabs6[:], oh[:, sl],
                                         start=True, stop=True,
                                         tile_position=(0, 32 * s))

            if O["qq_late"]:
                for s in range(SPC):
                    _qq_dma(s)
            # ---- post-scale + combine + divide (two halves, pipelined) ----
            E = sg.tile([128, W], F16, tag="E")
            nc.scalar.activation(E[:], QQ[:], ACTF.Exp, scale=sgn[:])
            SE = sg.tile([128, W], F16, tag="SE")
            with tc.tile_pool(name="v2p2", bufs=1, space="PSUM") as p2_pool:
                p2 = p2_pool.tile([36, W], F32)
                if O["tail"] == "halves":
                    for hf in range(2):
                        hsl = slice(hf * 1024, (hf + 1) * 1024)
                        nc.vector.tensor_mul(SE[:, hsl], pnd[:, hsl],
                                             E[:, hsl])
                        for n in range(2 * hf, 2 * hf + 2):
                            sl = slice(n * 512, (n + 1) * 512)
                            nc.tensor.matmul(p2[:, sl], cmb[:], SE[:, sl],
                                             start=True, stop=True)
                        nc.scalar.copy(numsb[:, hsl], p2[0:4, hsl])
                        nc.vector.reciprocal(dinv[:, hsl], p2[32:36, hsl])
                        nc.gpsimd.tensor_mul(h2sb[:, hsl], numsb[:, hsl],
                                             dinv[:, hsl])
                        if not O["ag_t"]:
                            nc.sync.dma_start(aps["h2_loc"].ap()[:, hsl],
                                              h2sb[:, hsl])
                else:
                    nc.vector.tensor_mul(SE[:], pnd[:], E[:])
                    for n in range(4):
                        sl = slice(n * 512, (n + 1) * 512)
                        nc.tensor.matmul(p2[:, sl], cmb[:], SE[:, sl],
                                         start=True, stop=True)
                    nc.vector.reciprocal(dinv[:], p2[32:36, :])
                    nc.vector.tensor_mul(h2sb[:], p2[0:4, :], dinv[:])
                    nc.sync.dma_start(aps["h2_loc"].ap(), h2sb[:])

        if O["ag_t"]:
            # transpose h2 locally, AllGather along partitions
            h2tl = sg.tile([128, PCH * SPC], F16)
            with tc.tile_pool(name="v2pt1", bufs=2, space="PSUM") as pt1_pool:
                for m in range(PCH):
                    pt1 = pt1_pool.tile([128, SPC], F16)
                    nc.tensor.transpose(pt1[:],
                                        h2sb[:, m * 128:(m + 1) * 128],
                                        eye4[:])
                    nc.vector.tensor_copy(
                        h2tl[:, m * SPC:(m + 1) * SPC], pt1[:])
            nc.sync.dma_start(aps["h2t_loc"].ap(), h2tl[:])
            nc.gpsimd.collective_compute(
                "AllGather", ALU.bypass, replica_groups=GROUPS,
                ins=[aps["h2t_loc"].ap()], outs=[aps["h2t_gat"].ap()])
            h2T = sg.tile([128, PCH, B], F16)
            h2Tv = h2T[:].rearrange("p m (e sl) -> p m e sl", sl=SPC)
            nc.sync.dma_start(
                h2Tv[:],
                _ap(aps["h2t_gat"], 0,
                    [[PCH * SPC, 128], [SPC, PCH],
                     [128 * PCH * SPC, NCORES], [1, SPC]]))
        else:
            nc.gpsimd.collective_compute(
                "AllGather", ALU.bypass, replica_groups=GROUPS,
                ins=[aps["h2_loc"].ap()], outs=[aps["h2_gat"].ap()])
            h2T = sg.tile([128, PCH, B], F16)
            h2f = sg.tile([B, W], F16, tag="h2f")
            nc.sync.dma_start(h2f[:], aps["h2_gat"].ap())
            with tc.tile_pool(name="v2pt2", bufs=2, space="PSUM") as pt2_pool:
                for m in range(PCH):
                    pt2 = pt2_pool.tile([128, B], F16)
                    nc.tensor.transpose(pt2[:],
                                        h2f[:, m * 128:(m + 1) * 128],
                                        eye32[:])
                    nc.vector.tensor_copy(h2T[:, m, :], pt2[:])
        sbo = sg.tile([B, FSL], F32)
        with tc.tile_pool(name="v2po", bufs=1, space="PSUM") as po_pool:
            pout = po_pool.tile([B, FSL], F32)
            for m in range(PCH):
                nc.tensor.matmul(pout[:], h2T[:, m, :], wpst[:, m, :],
                                 start=(m == 0), stop=(m == PCH - 1))
            nc.vector.tensor_add(sbo[:], pout[:], xb[:])
        nc.sync.dma_start(aps["out"].ap(), sbo[:])


def make_in_maps_v2(inputs):
    x = np.ascontiguousarray(np.asarray(inputs["x"], np.float32))
    Wq = np.asarray(inputs["Wq"], np.float32)
    Wk = np.asarray(inputs["Wk"], np.float32)
    Wv = np.asarray(inputs["Wv"], np.float32)
    Wp = np.asarray(inputs["Wp"], np.float32)
    bq = np.asarray(inputs["bq"], np.float32)
    bk = np.asarray(inputs["bk"], np.float32)
    bv = np.asarray(inputs["bv"], np.float32)
    bp = np.asarray(inputs["bp"], np.float32)
    in_maps = []
    for c in range(NCORES):
        cs = slice(c * FSL, (c + 1) * FSL)
        in_maps.append({
            "x": x,
            "wqkv": np.ascontiguousarray(np.concatenate(
                [Wq[:, cs], Wk[:, cs], Wv[:, cs]], axis=1).astype(np.float16)),
            "wqcs": np.ascontiguousarray(-np.concatenate(
                [Wq[:, cs], Wk[:, cs], Wv[:, cs]],
                axis=1).sum(0).astype(np.float16).reshape(1, -1)),
            "bqkv": np.ascontiguousarray(np.concatenate(
                [bq[cs], bk[cs], bv[cs]]).astype(np.float16).reshape(1, -1)),
            "wp": np.ascontiguousarray(Wp[:, cs].astype(np.float16)),
            "bp": np.ascontiguousarray(bp[cs]),
            "xs": np.ascontiguousarray(x[:, cs]),
        })
    return in_maps


_BUILT = {}


def _get_nc(mode, skip_gb=False):
    key = (mode, skip_gb)
    if key not in _BUILT:
        if mode == "v2":
            _BUILT[key] = build_v2()
        else:
            _BUILT[key] = build(mode, skip_gb=skip_gb)
    return _BUILT[key]


def make_in_maps(inputs):
    x = np.ascontiguousarray(np.asarray(inputs["x"], np.float32))
    gamma = np.ascontiguousarray(np.asarray(inputs["gamma"], np.float32))
    beta = np.ascontiguousarray(np.asarray(inputs["beta"], np.float32))
    Wq = np.asarray(inputs["Wq"], np.float32)
    Wk = np.asarray(inputs["Wk"], np.float32)
    Wv = np.asarray(inputs["Wv"], np.float32)
    Wp = np.asarray(inputs["Wp"], np.float32)
    bq = np.asarray(inputs["bq"], np.float32)
    bk = np.asarray(inputs["bk"], np.float32)
    bv = np.asarray(inputs["bv"], np.float32)
    bp = np.asarray(inputs["bp"], np.float32)
    in_maps = []
    for c in range(NCORES):
        cs = slice(c * FSL, (c + 1) * FSL)
        in_maps.append({
            "x": x,
            "gamma": gamma,
            "beta": beta,
            "wqkv": np.ascontiguousarray(
                np.concatenate([Wq[:, cs], Wk[:, cs], Wv[:, cs]], axis=1)),
            "bqkv": np.ascontiguousarray(
                np.concatenate([bq[cs], bk[cs], bv[cs]])),
            "wp": np.ascontiguousarray(Wp[:, cs]),
            "bp": np.ascontiguousarray(bp[cs]),
            "xs": np.ascontiguousarray(x[:, cs]),
        })
    return in_maps


def kernel(**inputs):
    skip_gb = bool(
        np.all(np.asarray(inputs["gamma"], np.float32) == 1.0)
        and np.all(np.asarray(inputs["beta"], np.float32) == 0.0))
    mode = MODE
    if mode == "v2" and not skip_gb:
        mode = "binned"  # v2 assumes gamma=1, beta=0
    nc = _get_nc(mode, skip_gb)
    if mode == "v2":
        in_maps = make_in_maps_v2(inputs)
    else:
        in_maps = make_in_maps(inputs)
    res = run_bass_kernel_spmd(nc, in_maps, core_ids=list(range(NCORES)))
    out = np.concatenate([res.results[c]["out"] for c in range(NCORES)],
                         axis=1)
    return np.ascontiguousarray(out.astype(np.float32))



# revision 51
# speedup vs baseline: 1.1780x; 1.1780x over previous
"""Trainium2 Bass kernel for nn_AttnBlock_12704513262242.

Math (per sample b, W=2048 "positions" with scalar q/k values):
  h   = layernorm(x) * gamma + beta
  q,k,v = h @ W* + b*
  attn  = softmax(-|q_j - k_i|, over i)
  h2[j] = sum_i attn[j,i] * v[i]
  out   = x + h2 @ Wp + bp

Sharding: feature-parallel QKV/proj (each core owns a 256-col slice of all
four weight matrices, host-cast to fp16), AllToAll to redistribute q/k/v
sample-major, data-parallel attention (4 samples per core), AllGather of
h2, feature-sliced output projection.  Host concatenates 8 [32,256] slices.

Default mode "v2" (the fast path; "naive"/"binned" are older fallbacks):
  * LayerNorm is deferred through the QKV matmul: matmul raw x.T (starts
    immediately, warms the PE pstate), stats via a selector matmul on a
    [128,512] view + Newton rsqrt on DVE (no ACT table switches), then
    qkv = rstd*(x@W + mu*(-colsum W) + std*b) via one K=2 rank-2 matmul
    and a scaling psum->sbuf copy.
  * Binned softmin attention: exp(-|q-k|) = e^{k-q} (k<=q) + e^{q-k} (k>q).
    Per sample: single is_ge prefix masks (DVE/Pool) + indicator matmuls
    accumulate prefix tables A,C,PB,PD at G=128 grid points; totals TB,TD
    are extracted by a row-selector matmul and appended as constant table
    columns; one UNSCALED nearest-bin one-hot per query feeds one eval
    matmul producing all 6 rows; all 4 samples land in one [128,W] psum at
    32-aligned offsets.  Post-scaling by e^{-+q} ([128,W] exp with a
    per-partition sign column), a 0/+-1 combine matmul, reciprocal and
    multiply finish num/den -> h2.  e^k tables are scaled by 1/64 (cancels
    in num/den) for fp16 range safety.
  * DMA queueing matters on HW: broadcasts/loads on the two HWDGE queues
    (SP + ACT), never gpsimd/SWDGE; weight fp16 preloads are spread and
    overlapped; attention constants load during the AllToAll.
"""

import os
import sys

import numpy as np

for _p in ("/opt/trn_rl_repo", "/root/.axon_site/_ro/trn_rl_repo"):
    if os.path.isdir(_p) and _p not in sys.path:
        sys.path.insert(0, _p)

import concourse.bass as bass
import concourse.tile as tile
from concourse import bacc, mybir
from concourse.bass_utils import run_bass_kernel_spmd

F32 = mybir.dt.float32
F16 = mybir.dt.float16
ALU = mybir.AluOpType
ACTF = mybir.ActivationFunctionType

B = 32            # batch
W = 2048          # width (positions / features)
NCORES = 8
PCH = W // 128    # 16 partition chunks of the feature dim
FSL = W // NCORES  # 256 feature-slice per core
QKVW = 3 * FSL    # 768
SPC = B // NCORES  # 4 samples per core

G = 128           # grid bins for binned mode
LO, HI = -8.0, 8.0
DELTA = (HI - LO) / (G - 1)
HALF = DELTA / 2.0
EPS = 1e-6

MODE = os.environ.get("ATTN_MODE", "v2")
GROUPS = [list(range(NCORES))]


def _ap(tensor_handle, offset, ap):
    return bass.AP(tensor=tensor_handle, offset=offset, ap=ap)


def build(mode=None, reps=1, skip_gb=False, fake_cc=False,
          ohm_eng="dve", oh_bufs=2, mm16="dve", cc16=True, abl="full"):
    mode = mode or MODE
    nc = bacc.Bacc("TRN2", target_bir_lowering=False, debug=False,
                   num_devices=NCORES)

    x_t = nc.dram_tensor("x", [B, W], F32, kind="ExternalInput")
    gamma_t = nc.dram_tensor("gamma", [W], F32, kind="ExternalInput")
    beta_t = nc.dram_tensor("beta", [W], F32, kind="ExternalInput")
    wqkv_t = nc.dram_tensor("wqkv", [W, QKVW], F32, kind="ExternalInput")
    bqkv_t = nc.dram_tensor("bqkv", [QKVW], F32, kind="ExternalInput")
    wp_t = nc.dram_tensor("wp", [W, FSL], F32, kind="ExternalInput")
    bp_t = nc.dram_tensor("bp", [FSL], F32, kind="ExternalInput")
    xs_t = nc.dram_tensor("xs", [B, FSL], F32, kind="ExternalInput")
    out_t = nc.dram_tensor("out", [B, FSL], F32, kind="ExternalOutput")

    ccdt = F16 if cc16 else F32
    qkv_loc = nc.dram_tensor("qkv_loc", [B, QKVW], ccdt)
    qkv_a2a = nc.dram_tensor("qkv_a2a", [B, QKVW], ccdt)
    h2_loc = nc.dram_tensor("h2_loc", [SPC, W], ccdt)
    h2_gat = nc.dram_tensor("h2_gat", [B, W], ccdt, addr_space="Shared")

    c_eye32 = nc.inline_tensor(np.eye(32, dtype=np.float32), "c_eye32")
    c_eye8 = nc.inline_tensor(np.eye(8, dtype=np.float16), "c_eye8")
    c_eye8f = nc.inline_tensor(np.eye(8, dtype=np.float32), "c_eye8f")
    c_eye2 = nc.inline_tensor(np.eye(2, dtype=np.float32), "c_eye2")
    c_eye32_16 = nc.inline_tensor(np.eye(32, dtype=np.float16), "c_eye32_16")
    c_ones132 = nc.inline_tensor(np.ones((1, 32), np.float32), "c_ones132")
    gridv = np.linspace(LO, HI, G, dtype=np.float64).astype(np.float32)
    c_gcol = nc.inline_tensor(gridv.reshape(G, 1), "c_gcol")
    c_gcoln = nc.inline_tensor(-gridv.reshape(G, 1), "c_gcoln")
    c_grow = nc.inline_tensor(gridv.reshape(1, G), "c_grow")

    aps = dict(
        x=x_t.ap(), gamma=gamma_t.ap(), beta=beta_t.ap(),
        wqkv=wqkv_t.ap(), bqkv=bqkv_t.ap(), wp=wp_t.ap(), bp=bp_t.ap(),
        xs=xs_t.ap(), out=out_t.ap(),
        qkv_loc=qkv_loc.ap(), qkv_a2a=qkv_a2a.ap(),
        h2_loc=h2_loc.ap(), h2_gat=h2_gat.ap(),
        eye32=c_eye32.ap(), eye32_16=c_eye32_16.ap(),
        eye8=c_eye8.ap(), eye8f32=c_eye8f.ap(), eye2=c_eye2.ap(),
        ones132=c_ones132.ap(), gcol=c_gcol.ap(), gcoln=c_gcoln.ap(),
        grow=c_grow.ap(),
        a2a_tensor=qkv_a2a,
    )

    aps["fake_cc"] = fake_cc
    aps["ohm_eng"] = ohm_eng
    aps["oh_bufs"] = oh_bufs
    aps["mm16"] = mm16
    aps["cc16"] = cc16
    aps["abl"] = abl
    with tile.TileContext(nc) as tc:
        for _rep in range(reps):
            _build_tile(tc, aps, mode, skip_gb)

    nc.compile()
    return nc


def _build_tile(tc, aps, mode, skip_gb=False):
    nc = tc.nc

    with tc.tile_pool(name="singles", bufs=1) as singles:
        # ---- constants into SBUF ----
        eye32 = singles.tile([32, 32], F32)
        nc.sync.dma_start(eye32[:], aps["eye32"])
        eye32_16 = singles.tile([32, 32], F16)
        nc.sync.dma_start(eye32_16[:], aps["eye32_16"])
        eye8 = singles.tile([8, 8], F16 if aps["cc16"] else F32)
        nc.sync.dma_start(eye8[:], aps["eye8"]
                          if aps["cc16"] else aps["eye8f32"])
        eye2 = singles.tile([2, 2], F32)
        nc.sync.dma_start(eye2[:], aps["eye2"])
        ones132 = singles.tile([1, 32], F32)
        nc.sync.dma_start(ones132[:], aps["ones132"])
        gcol = singles.tile([G, 1], F32)
        nc.sync.dma_start(gcol[:], aps["gcol"])
        gcoln = singles.tile([G, 1], F32)
        nc.sync.dma_start(gcoln[:], aps["gcoln"])
        gbc = singles.tile([128, G], F32)
        nc.gpsimd.dma_start(gbc[:], aps["grow"].partition_broadcast(128))

        # ---- small weight bits ----
        bq32 = singles.tile([1, QKVW], F32)
        nc.sync.dma_start(bq32[:], aps["bqkv"].partition_broadcast(1))

        # residual + bp, exact fp32: xb = x_slice + bp
        xb = singles.tile([B, FSL], F32)
        bpb = singles.tile([B, FSL], F32)
        nc.gpsimd.dma_start(bpb[:], aps["bp"].partition_broadcast(B))
        xsl = singles.tile([B, FSL], F32)
        nc.sync.dma_start(xsl[:], aps["xs"])
        nc.vector.tensor_add(xb[:], xsl[:], bpb[:])

        # ---- layernorm (replicated, all 32 samples) ----
        sbx = singles.tile([B, W], F32, tag="bigio")
        nc.sync.dma_start(sbx[:], aps["x"])
        xg = sbx[:].rearrange("b (s f) -> b s f", s=4)  # 4 subgroups of 512
        stats = singles.tile([B, 4, 6], F32)
        for sg in range(4):
            nc.vector.bn_stats(stats[:, sg, :], xg[:, sg, :])
        mv = singles.tile([B, 2], F32)
        nc.vector.bn_aggr(mv[:], stats[:])
        eps_t = singles.tile([B, 1], F32)
        nc.vector.memset(eps_t[:], EPS)
        stdv = singles.tile([B, 1], F32)
        nc.scalar.activation(stdv[:], mv[:, 1:2], ACTF.Sqrt, bias=eps_t[:])
        rstd = singles.tile([B, 1], F32)
        nc.vector.reciprocal(rstd[:], stdv[:])
        h = singles.tile([B, W], F32)
        nc.vector.tensor_scalar(h[:], sbx[:], mv[:, 0:1], rstd[:],
                                op0=ALU.subtract, op1=ALU.mult)
        if not skip_gb:
            gb = singles.tile([B, W], F32, tag="gbb")
            nc.gpsimd.dma_start(gb[:], aps["gamma"].partition_broadcast(B))
            nc.vector.tensor_mul(h[:], h[:], gb[:])
            bb = singles.tile([B, W], F32, tag="gbb")
            nc.gpsimd.dma_start(bb[:], aps["beta"].partition_broadcast(B))
            nc.vector.tensor_add(h[:], h[:], bb[:])

        # ---- transpose h -> hT [128, PCH, 32] ----
        mm16 = aps["mm16"]
        wdt = F16 if mm16 != "off" else F32
        hT = singles.tile([128, PCH, B], wdt)
        with tc.tile_pool(name="ptr", bufs=2, space="PSUM") as ptr_pool:
            for ci in range(PCH):
                ptr = ptr_pool.tile([128, B], F32)
                nc.tensor.transpose(ptr[:], h[:, ci * 128:(ci + 1) * 128],
                                    eye32[:])
                nc.vector.tensor_copy(hT[:, ci, :], ptr[:])

        # ---- qkv matmul: [32, 768] = h @ wqkv + bqkv ----
        sbq = singles.tile([B, QKVW], F16 if aps["cc16"] else F32)
        with (
            tc.tile_pool(name="pq", bufs=1, space="PSUM") as pq_pool,
            tc.tile_pool(name="wst", bufs=4) as wst_pool,
        ):
            pq = pq_pool.tile([B, QKVW], F32)
            for ci in range(PCH):
                wch = wst_pool.tile([128, QKVW], F32, tag="wch")
                nc.sync.dma_start(wch[:],
                                  aps["wqkv"][ci * 128:(ci + 1) * 128, :])
                if mm16 == "off":
                    wmm = wch
                else:
                    wmm = wst_pool.tile([128, QKVW], F16, tag="wch16")
                    nc.vector.tensor_copy(wmm[:], wch[:])
                nc.tensor.matmul(pq[:, 0:512], hT[:, ci, :],
                                 wmm[:, 0:512],
                                 start=(ci == 0), stop=False)
                nc.tensor.matmul(pq[:, 512:QKVW], hT[:, ci, :],
                                 wmm[:, 512:QKVW],
                                 start=(ci == 0), stop=False)
            nc.tensor.matmul(pq[:, 0:512], ones132[:], bq32[:, 0:512],
                             start=False, stop=True)
            nc.tensor.matmul(pq[:, 512:QKVW], ones132[:], bq32[:, 512:QKVW],
                             start=False, stop=True)
            nc.vector.tensor_copy(sbq[:], pq[:])
        nc.sync.dma_start(aps["qkv_loc"], sbq[:])

        if aps.get("fake_cc"):
            nc.sync.dma_start(aps["qkv_a2a"], aps["qkv_loc"])
        else:
            nc.gpsimd.collective_compute(
                "AllToAll", ALU.bypass, replica_groups=GROUPS,
                ins=[aps["qkv_loc"]], outs=[aps["qkv_a2a"]])

        # ---- attention (4 samples) ----
        abl = aps["abl"]
        num_t = singles.tile([SPC, W], F32)
        den_t = singles.tile([SPC, W], F32)
        shared = dict(a2a=aps["a2a_tensor"], num=num_t, den=den_t,
                      eye8=eye8, eye2=eye2, gbc=gbc, gcol=gcol,
                      gcoln=gcoln, ohm_eng=aps["ohm_eng"],
                      oh_bufs=aps["oh_bufs"],
                      ccdt=F16 if aps["cc16"] else F32)
        if abl in ("no_attn", "qkv_only"):
            nc.vector.memset(num_t[:], 1.0)
            nc.vector.memset(den_t[:], 1.0)
        elif mode == "binned":
            _attn_binned(tc, shared)
        else:
            _attn_naive(tc, shared)

        dinv = singles.tile([SPC, W], F32)
        nc.vector.reciprocal(dinv[:], den_t[:])
        sbh2 = singles.tile([SPC, W], F16 if aps["cc16"] else F32)
        nc.vector.tensor_mul(sbh2[:], num_t[:], dinv[:])
        nc.sync.dma_start(aps["h2_loc"], sbh2[:])

        if abl in ("no_proj", "qkv_only"):
            nc.sync.dma_start(aps["out"], xb[:])
            return
        if aps.get("fake_cc"):
            nc.sync.dma_start(aps["h2_gat"][0:SPC, :], aps["h2_loc"])
        else:
            nc.gpsimd.collective_compute(
                "AllGather", ALU.bypass, replica_groups=GROUPS,
                ins=[aps["h2_loc"]], outs=[aps["h2_gat"]])

        # ---- output projection ----
        h2dt = F16 if aps["cc16"] else F32
        h2f = singles.tile([B, W], h2dt, tag="bigio2")
        nc.sync.dma_start(h2f[:], aps["h2_gat"])
        h2T = singles.tile([128, PCH, B], wdt)
        eyeh2 = eye32_16 if aps["cc16"] else eye32
        with tc.tile_pool(name="ptr2", bufs=2, space="PSUM") as ptr2_pool:
            for ci in range(PCH):
                ptr2 = ptr2_pool.tile([128, B], h2dt)
                nc.tensor.transpose(ptr2[:], h2f[:, ci * 128:(ci + 1) * 128],
                                    eyeh2[:])
                nc.vector.tensor_copy(h2T[:, ci, :], ptr2[:])

        sbo = singles.tile([B, FSL], F32)
        with (
            tc.tile_pool(name="pout", bufs=1, space="PSUM") as pout_pool,
            tc.tile_pool(name="wpst", bufs=4) as wpst_pool,
        ):
            pout = pout_pool.tile([B, FSL], F32)
            for ci in range(PCH):
                wpch = wpst_pool.tile([128, FSL], F32, tag="wpch")
                nc.sync.dma_start(wpch[:],
                                  aps["wp"][ci * 128:(ci + 1) * 128, :])
                if mm16 == "off":
                    wpmm = wpch
                else:
                    wpmm = wpst_pool.tile([128, FSL], F16, tag="wpch16")
                    nc.vector.tensor_copy(wpmm[:], wpch[:])
                nc.tensor.matmul(pout[:], h2T[:, ci, :], wpmm[:],
                                 start=(ci == 0), stop=(ci == PCH - 1))
            nc.vector.tensor_add(sbo[:], pout[:], xb[:])
        nc.sync.dma_start(aps["out"], sbo[:])


def _load_qkv_sample(nc, kv_pool, ptp_pool, shared, s):
    """Per-sample loads from the AllToAll result: broadcast q [128, W] and
    k/v transposed into [128, 16] (feature chunk m = half*8 + coreblk)."""
    a2a = shared["a2a"]
    eye8 = shared["eye8"]
    cdt = shared["ccdt"]
    dma = nc.sync.dma_start if cdt == F16 else nc.gpsimd.dma_start
    row_k = kv_pool.tile([8, 256], cdt, tag="krow")
    dma(row_k[:], _ap(a2a, s * QKVW + FSL, [[4 * QKVW, 8], [1, 256]]))
    row_v = kv_pool.tile([8, 256], cdt, tag="vrow")
    dma(row_v[:], _ap(a2a, s * QKVW + 2 * FSL, [[4 * QKVW, 8], [1, 256]]))
    kTt = kv_pool.tile([128, PCH], F32, tag="kT")
    vTt = kv_pool.tile([128, PCH], F32, tag="vT")
    for half in range(2):
        ptk = ptp_pool.tile([128, 8], cdt, tag="ptp")
        nc.tensor.transpose(ptk[:], row_k[:, half * 128:(half + 1) * 128],
                            eye8[:])
        nc.vector.tensor_copy(kTt[:, half * 8:(half + 1) * 8], ptk[:])
        ptv = ptp_pool.tile([128, 8], cdt, tag="ptp")
        nc.tensor.transpose(ptv[:], row_v[:, half * 128:(half + 1) * 128],
                            eye8[:])
        nc.vector.tensor_copy(vTt[:, half * 8:(half + 1) * 8], ptv[:])
    return kTt, vTt


def _q_broadcast(nc, pool, shared, s, clamp):
    qb = pool.tile([128, W], shared["ccdt"], tag="qb")
    src = _ap(shared["a2a"], s * QKVW, [[0, 128], [4 * QKVW, 8], [1, 256]])
    if shared["ccdt"] == F16:
        nc.sync.dma_start(qb[:], src)
    else:
        nc.gpsimd.dma_start(qb[:], src)
    if clamp:
        nc.vector.tensor_scalar(qb[:], qb[:], LO, HI,
                                op0=ALU.max, op1=ALU.min)
    return qb


def _attn_binned(tc, shared):
    nc = tc.nc
    gbc = shared["gbc"]
    gcoln = shared["gcoln"]
    eye2 = shared["eye2"]
    ohm_op = (nc.gpsimd.tensor_mul if shared["ohm_eng"] == "gpsimd"
              else nc.vector.tensor_mul)
    with (
        tc.tile_pool(name="akv", bufs=2) as kv_pool,
        tc.tile_pool(name="aqb", bufs=2) as qb_pool,
        tc.tile_pool(name="aoh", bufs=shared["oh_bufs"]) as oh_pool,
        tc.tile_pool(name="amk", bufs=3) as mk_pool,
        tc.tile_pool(name="atab", bufs=2) as tab_pool,
        tc.tile_pool(name="ptp", bufs=2, space="PSUM") as ptp_pool,
        tc.tile_pool(name="ptab", bufs=2, space="PSUM") as ptab_pool,
        tc.tile_pool(name="pnd", bufs=1, space="PSUM") as pnd_pool,
    ):
        for s in range(SPC):
            qb = _q_broadcast(nc, qb_pool, shared, s, clamp=False)
            kTt, vTt = _load_qkv_sample(nc, kv_pool, ptp_pool, shared, s)

            ek = kv_pool.tile([128, PCH], F32, tag="ek")
            nc.scalar.activation(ek[:], kTt[:], ACTF.Exp)
            emk = kv_pool.tile([128, PCH], F32, tag="emk")
            nc.scalar.activation(emk[:], kTt[:], ACTF.Exp, scale=-1.0)
            u = kv_pool.tile([128, PCH, 4], F16, tag="u")
            nc.vector.tensor_mul(u[:, :, 0], ek[:], vTt[:])
            nc.vector.tensor_copy(u[:, :, 1], ek[:])
            nc.vector.tensor_mul(u[:, :, 2], emk[:], vTt[:])
            nc.vector.tensor_copy(u[:, :, 3], emk[:])

            # cumulative tables at the G grid points: psum rows = u-type
            ptab = ptab_pool.tile([4, 2 * G], F32, tag="ptab")
            for m in range(PCH):
                mk = mk_pool.tile([128, 2 * G], F16, tag="mk")
                nc.vector.tensor_scalar(mk[:, 0:G], gbc[:],
                                        kTt[:, m:m + 1], None, op0=ALU.is_ge)
                nc.vector.tensor_scalar(mk[:, G:2 * G], gbc[:],
                                        kTt[:, m:m + 1], None, op0=ALU.is_lt)
                nc.tensor.matmul(ptab[:], u[:, m, :], mk[:],
                                 start=(m == 0), stop=(m == PCH - 1))
            # rows 0,1 x cols [0,G)  = A,C (prefix with e^k);
            # rows 2,3 x cols [G,2G) = B,D (suffix with e^-k)
            sbtab = tab_pool.tile([4, 2 * G], F32, tag="sbtab")
            nc.scalar.copy(sbtab[:], ptab[:])
            sbBD = tab_pool.tile([2, G], F32, tag="sbBD")
            nc.sync.dma_start(sbBD[:], sbtab[2:4, G:2 * G])
            tabs = tab_pool.tile([G, 4], F16, tag="tabs")
            ptt = ptp_pool.tile([G, 2], F32, tag="ptp")
            nc.tensor.transpose(ptt[:], sbtab[0:2, 0:G], eye2[:])
            nc.vector.tensor_copy(tabs[:, 0:2], ptt[:])
            ptt2 = ptp_pool.tile([G, 2], F32, tag="ptp")
            nc.tensor.transpose(ptt2[:], sbBD[:], eye2[:])
            nc.vector.tensor_copy(tabs[:, 2:4], ptt2[:])

            # one-hot of nearest grid point, pre-scaled by e^{-+q}
            t1 = qb_pool.tile([128, W], F32, tag="t1", bufs=2)
            nc.scalar.activation(t1[:], qb[:], ACTF.Abs, bias=gcoln[:])
            oh = oh_pool.tile([128, W], F16, tag="oh")
            nc.vector.tensor_scalar(oh[:], t1[:], HALF, None, op0=ALU.is_le)
            emq = oh_pool.tile([128, W], F16, tag="emq")
            nc.scalar.activation(emq[:], qb[:], ACTF.Exp, scale=-1.0)
            epq = oh_pool.tile([128, W], F16, tag="epq")
            nc.scalar.activation(epq[:], qb[:], ACTF.Exp, scale=1.0)
            ohm = oh_pool.tile([128, W], F16, tag="ohm")
            ohm_op(ohm[:], oh[:], emq[:])
            ohp = oh_pool.tile([128, W], F16, tag="ohp")
            ohm_op(ohp[:], oh[:], epq[:])

            pnd = pnd_pool.tile([2, W], F32, tag="pnd")
            for n in range(4):
                sl = slice(n * 512, (n + 1) * 512)
                nc.tensor.matmul(pnd[:, sl], tabs[:, 0:2], ohm[:, sl],
                                 start=True, stop=False)
                nc.tensor.matmul(pnd[:, sl], tabs[:, 2:4], ohp[:, sl],
                                 start=False, stop=True)
            ns_s = oh_pool.tile([2, W], F32, tag="ns")
            nc.scalar.copy(ns_s[:], pnd[:])
            nc.sync.dma_start(shared["num"][s:s + 1, :], ns_s[0:1, :])
            nc.sync.dma_start(shared["den"][s:s + 1, :], ns_s[1:2, :])


def _attn_naive(tc, shared):
    nc = tc.nc
    with (
        tc.tile_pool(name="akv", bufs=2) as kv_pool,
        tc.tile_pool(name="aqb", bufs=2) as qb_pool,
        tc.tile_pool(name="aab", bufs=2) as ab_pool,
        tc.tile_pool(name="apt", bufs=3) as pt_pool,
        tc.tile_pool(name="ptp", bufs=2, space="PSUM") as ptp_pool,
        tc.tile_pool(name="pnd", bufs=1, space="PSUM") as pnd_pool,
    ):
        for s in range(SPC):
            qb = _q_broadcast(nc, qb_pool, shared, s, clamp=False)
            kTt, vTt = _load_qkv_sample(nc, kv_pool, ptp_pool, shared, s)

            nk = kv_pool.tile([128, PCH], F32, tag="nk")
            nc.vector.tensor_scalar(nk[:], kTt[:], -1.0, None, op0=ALU.mult)
            u2 = kv_pool.tile([128, PCH, 2], F16, tag="u2")
            nc.vector.tensor_copy(u2[:, :, 0], vTt[:])
            nc.vector.memset(u2[:, :, 1], 1.0)

            pnd = pnd_pool.tile([2, W], F32, tag="pnd")
            for m in range(PCH):
                ab = ab_pool.tile([128, W], F32, tag="ab")
                nc.scalar.activation(ab[:], qb[:], ACTF.Abs,
                                     bias=nk[:, m:m + 1])
                pt = pt_pool.tile([128, W], F16, tag="pt")
                nc.scalar.activation(pt[:], ab[:], ACTF.Exp, scale=-1.0)
                for n in range(4):
                    sl = slice(n * 512, (n + 1) * 512)
                    nc.tensor.matmul(pnd[:, sl], u2[:, m, :], pt[:, sl],
                                     start=(m == 0), stop=(m == PCH - 1))
            ns_s = ab_pool.tile([2, W], F32, tag="ns")
            nc.scalar.copy(ns_s[:], pnd[:])
            nc.sync.dma_start(shared["num"][s:s + 1, :], ns_s[0:1, :])
            nc.sync.dma_start(shared["den"][s:s + 1, :], ns_s[1:2, :])


# ---------------------------------------------------------------------------
# v2: restructured kernel.
#   LN stats via matmul on [128,512] layout; fp16 weights (host-cast);
#   binned attention with prefix-only masks, shared unscaled one-hot,
#   batched post-scaling, all num/den in one PSUM tile; feature-par proj.
# ---------------------------------------------------------------------------

LN64 = float(np.log(64.0))


def _v2_consts(nc):
    G_ = G
    gridv = np.linspace(LO, HI, G_, dtype=np.float64).astype(np.float32)
    c = {}
    c["eye8"] = nc.inline_tensor(np.eye(8, dtype=np.float16), "v2_eye8")
    c["eye4"] = nc.inline_tensor(np.eye(4, dtype=np.float16), "v2_eye4")
    rs = np.zeros((128, G_), np.float16)
    rs[G_ - 1, :] = 1.0
    c["rowsel"] = nc.inline_tensor(rs, "v2_rowsel")
    c["eye32"] = nc.inline_tensor(np.eye(32, dtype=np.float16), "v2_eye32")
    c["eye128"] = nc.inline_tensor(np.eye(128, dtype=np.float16), "v2_eye128")
    c["eye128f"] = nc.inline_tensor(np.eye(128, dtype=np.float32),
                                    "v2_eye128f")
    c["ones1"] = nc.inline_tensor(np.ones((1, 32), np.float16), "v2_ones1")
    c["gcoln"] = nc.inline_tensor(-gridv.reshape(G_, 1), "v2_gcoln")
    c["gcol"] = nc.inline_tensor(gridv.reshape(G_, 1), "v2_gcol")
    c["gpH"] = nc.inline_tensor(
        (gridv + np.float32(HALF)).reshape(G_, 1), "v2_gpH")
    c["gmH"] = nc.inline_tensor(
        (gridv - np.float32(HALF)).reshape(G_, 1), "v2_gmH")
    c["gbc16"] = nc.inline_tensor(
        np.tile(gridv.reshape(1, G_), (128, 1)).astype(np.float16), "v2_gbc16")
    # stats: selT[p, s] = 1 if p//4 == s  (x128 partition p = s*4 + c)
    selT = np.zeros((128, 32), np.float16)
    for p in range(128):
        selT[p, p // 4] = 1.0
    c["selT"] = nc.inline_tensor(selT, "v2_selT")
    # sign column for E = exp(sgn * q): rows (A,C -> -1), (B,D -> +1)
    sgn = np.ones((128, 1), np.float32)
    for s in range(4):
        sgn[32 * s + 0] = -1.0
        sgn[32 * s + 1] = -1.0
    c["sgn"] = nc.inline_tensor(sgn, "v2_sgn")
    # num_s = emq*A - epq*PB + epq*TB ; den_s = emq*C - epq*PD + epq*TD
    cmb = np.zeros((128, 36), np.float16)
    for s in range(4):
        cmb[32 * s + 0, s] = 1.0
        cmb[32 * s + 2, s] = -1.0
        cmb[32 * s + 4, s] = 1.0
        cmb[32 * s + 1, 32 + s] = 1.0
        cmb[32 * s + 3, 32 + s] = -1.0
        cmb[32 * s + 5, 32 + s] = 1.0
    c["cmb"] = nc.inline_tensor(cmb, "v2_cmb")
    return c


def build_v2(reps=1, opts=None):
    nc = bacc.Bacc("TRN2", target_bir_lowering=False, debug=False,
                   num_devices=NCORES)

    x_t = nc.dram_tensor("x", [B, W], F32, kind="ExternalInput")
    wqkv_t = nc.dram_tensor("wqkv", [W, QKVW], F16, kind="ExternalInput")
    wqcs_t = nc.dram_tensor("wqcs", [1, QKVW], F16, kind="ExternalInput")
    bqkv_t = nc.dram_tensor("bqkv", [1, QKVW], F16, kind="ExternalInput")
    wp_t = nc.dram_tensor("wp", [W, FSL], F16, kind="ExternalInput")
    bp_t = nc.dram_tensor("bp", [FSL], F32, kind="ExternalInput")
    xs_t = nc.dram_tensor("xs", [B, FSL], F32, kind="ExternalInput")
    out_t = nc.dram_tensor("out", [B, FSL], F32, kind="ExternalOutput")

    qkv_loc = nc.dram_tensor("qkv_loc", [B, QKVW], F16)
    qkv_a2a = nc.dram_tensor("qkv_a2a", [B, QKVW], F16)
    h2_loc = nc.dram_tensor("h2_loc", [SPC, W], F16)
    h2_gat = nc.dram_tensor("h2_gat", [B, W], F16, addr_space="Shared")
    h2t_loc = nc.dram_tensor("h2t_loc", [128, PCH * SPC], F16)
    h2t_gat = nc.dram_tensor("h2t_gat", [128 * NCORES, PCH * SPC], F16,
                             addr_space="Shared")

    consts = _v2_consts(nc)

    aps = dict(
        x=x_t, wqkv=wqkv_t, wqcs=wqcs_t, bqkv=bqkv_t, wp=wp_t, bp=bp_t,
        xs=xs_t,
        out=out_t, qkv_loc=qkv_loc, qkv_a2a=qkv_a2a,
        h2_loc=h2_loc, h2_gat=h2_gat, h2t_loc=h2t_loc, h2t_gat=h2t_gat,
        consts=consts,
    )
    aps["opts"] = dict(qb_eng="sync", qq_eng="sync", kv_eng="scalar",
                       masks="prebuild", tail="halves",
                       hT="pe", h2T="pe", stats_sq="pool", newton=True,
                       wdma="mix", qq_late=True, warm=0,
                       t1_eng="act", mask_dve_mod=4, ag_t=False)
    aps["opts"].update(opts or {})
    with tile.TileContext(nc) as tc:
        for _rep in range(reps):
            _build_tile_v2(tc, aps)
    nc.compile()
    return nc


def _build_tile_v2(tc, aps):
    nc = tc.nc
    C = aps["consts"]
    O = aps["opts"]

    with tc.tile_pool(name="v2s", bufs=1) as sg:
        # ---- input DMAs first (critical path) ----
        x128 = sg.tile([128, 512], F32)
        nc.sync.dma_start(x128[:], _ap(aps["x"], 0, [[512, 128], [1, 512]]))
        selT = sg.tile([128, 32], F16)
        nc.scalar.dma_start(selT[:], C["selT"].ap())
        eye128f = sg.tile([128, 128], F32)
        nc.scalar.dma_start(eye128f[:], C["eye128f"].ap())
        xsl = sg.tile([B, FSL], F32)
        nc.sync.dma_start(xsl[:], aps["xs"].ap())
        # LN inputs squared + f16 copy, ahead of any weight traffic
        xf = sg.tile([128, 1024], F16)
        nc.vector.tensor_copy(xf[:, 0:512], x128[:])
        if O["stats_sq"] == "pool":
            nc.gpsimd.tensor_mul(xf[:, 512:1024], x128[:], x128[:])
        else:
            nc.scalar.activation(xf[:, 512:1024], x128[:], ACTF.Square)
        wcb = sg.tile([2, QKVW], F16)
        nc.sync.dma_start(wcb[0:1, :], aps["wqcs"].ap())
        nc.sync.dma_start(wcb[1:2, :], aps["bqkv"].ap())
        eye32 = sg.tile([32, 32], F16)
        nc.scalar.dma_start(eye32[:], C["eye32"].ap())
        ones1 = sg.tile([1, 32], F16)
        nc.sync.dma_start(ones1[:], C["ones1"].ap())
        bq16 = sg.tile([1, QKVW], F16)
        nc.scalar.dma_start(bq16[:], aps["bqkv"].ap())
        mln64 = sg.tile([128, 1], F32)
        nc.vector.memset(mln64[:], -LN64)

        # ---- weights (fp16, preloaded; spread across queues) ----
        if O["wdma"] == "hwdge":
            wengs = (nc.scalar, nc.sync)
        else:
            wengs = (nc.scalar, nc.gpsimd)
        wst = sg.tile([128, PCH, QKVW], F16, tag="wst")
        for i in range(8):
            src = _ap(aps["wqkv"], i * 2 * 128 * QKVW,
                      [[QKVW, 128], [128 * QKVW, 2], [1, QKVW]])
            wengs[i % 2].dma_start(wst[:, 2 * i:2 * i + 2, :], src)
        wpst = sg.tile([128, PCH, FSL], F16, tag="wpst")
        for i in range(4):
            src = _ap(aps["wp"], i * 4 * 128 * FSL,
                      [[FSL, 128], [128 * FSL, 4], [1, FSL]])
            wengs[i % 2].dma_start(wpst[:, 4 * i:4 * i + 4, :], src)

        # residual + bp: xb = x_slice + bp
        xb = sg.tile([B, FSL], F32)
        bpb = sg.tile([B, FSL], F32)
        beng = nc.scalar if O["wdma"] == "hwdge" else nc.gpsimd
        beng.dma_start(bpb[:], aps["bp"].ap().partition_broadcast(B))
        nc.vector.tensor_add(xb[:], xsl[:], bpb[:])

        # ---- layernorm stats on [128, 512] layout ----
        with tc.tile_pool(name="v2pst", bufs=1, space="PSUM") as pst_pool:
            pstat = pst_pool.tile([32, 1024], F32)
            nc.tensor.matmul(pstat[:, 0:512], selT[:], xf[:, 0:512],
                             start=True, stop=True)
            nc.tensor.matmul(pstat[:, 512:1024], selT[:], xf[:, 512:1024],
                             start=True, stop=True)
            reds = sg.tile([32, 2], F32)
            nc.vector.tensor_reduce(reds[:, 0:1], pstat[:, 0:512],
                                    axis=mybir.AxisListType.X,
                                    op=ALU.add)
            nc.vector.tensor_reduce(reds[:, 1:2], pstat[:, 512:1024],
                                    axis=mybir.AxisListType.X,
                                    op=ALU.add)
        m2 = sg.tile([32, 2], F32)
        nc.vector.tensor_scalar(m2[:], reds[:], 1.0 / W, None, op0=ALU.mult)
        musq = sg.tile([32, 1], F32)
        nc.vector.tensor_mul(musq[:], m2[:, 0:1], m2[:, 0:1])
        var = sg.tile([32, 1], F32)
        nc.vector.tensor_sub(var[:], m2[:, 1:2], musq[:])
        # rstd via Newton rsqrt (var ~ 1, 3 iterations, all on DVE)
        y1 = sg.tile([32, 1], F32)
        nc.vector.tensor_scalar(y1[:], var[:], -0.5, 1.5,
                                op0=ALU.mult, op1=ALU.add)
        t_n = sg.tile([32, 4], F32, tag="newt")
        nc.vector.tensor_mul(t_n[:, 0:1], y1[:], y1[:])
        nc.vector.tensor_mul(t_n[:, 1:2], t_n[:, 0:1], var[:])
        nc.vector.tensor_scalar(t_n[:, 2:3], t_n[:, 1:2], -0.5, 1.5,
                                op0=ALU.mult, op1=ALU.add)
        y2 = sg.tile([32, 1], F32)
        nc.vector.tensor_mul(y2[:], y1[:], t_n[:, 2:3])
        t_m = sg.tile([32, 4], F32, tag="newt")
        nc.vector.tensor_mul(t_m[:, 0:1], y2[:], y2[:])
        nc.vector.tensor_mul(t_m[:, 1:2], t_m[:, 0:1], var[:])
        nc.vector.tensor_scalar(t_m[:, 2:3], t_m[:, 1:2], -0.5, 1.5,
                                op0=ALU.mult, op1=ALU.add)
        rstd32 = sg.tile([32, 1], F32)
        nc.vector.tensor_mul(rstd32[:], y2[:], t_m[:, 2:3])
        # mu_std cols (mu, std);  std = var * rstd = sqrt(var)
        mu_std = sg.tile([32, 2], F16)
        nc.vector.tensor_copy(mu_std[:, 0:1], m2[:, 0:1])
        nc.vector.tensor_mul(mu_std[:, 1:2], var[:], rstd32[:])
        musrow = sg.tile([2, 32], F16)
        with tc.tile_pool(name="v2pmu", bufs=1, space="PSUM") as pmu_pool:
            pmu = pmu_pool.tile([2, 32], F32)
            nc.tensor.matmul(pmu[:], mu_std[:], eye32[:],
                             start=True, stop=True)
            nc.vector.tensor_copy(musrow[:], pmu[:])

        # ---- transpose raw x (f32) -> xT [128, PCH, 32] (f16) ----
        hT = sg.tile([128, PCH, B], F16)
        hTv = hT[:].rearrange("p (c b) s -> p b c s", b=4)
        with tc.tile_pool(name="v2ptr", bufs=2, space="PSUM") as ptr_pool:
            for bb in range(4):
                ptr = ptr_pool.tile([128, 128], F32)
                nc.tensor.transpose(ptr[:],
                                    x128[:, bb * 128:(bb + 1) * 128],
                                    eye128f[:])
                ptr_r = ptr[:].rearrange("p (s c) -> p c s", c=4)
                nc.vector.tensor_copy(hTv[:, bb, :, :], ptr_r[:])

        # ---- qkv matmul on raw x; LN folded in afterwards ----
        # qkv = rstd*(xq + mu*(-S) + std*b)  with S = colsum(wqkv)
        sbq = sg.tile([B, QKVW], F16)
        with tc.tile_pool(name="v2pq", bufs=1, space="PSUM") as pq_pool:
            pq = pq_pool.tile([B, QKVW], F32)
            for m in range(PCH):
                nc.tensor.matmul(pq[:, 0:512], hT[:, m, :], wst[:, m, 0:512],
                                 start=(m == 0), stop=False)
                nc.tensor.matmul(pq[:, 512:QKVW], hT[:, m, :],
                                 wst[:, m, 512:QKVW],
                                 start=(m == 0), stop=False)
            for sl in (slice(0, 512), slice(512, QKVW)):
                nc.tensor.matmul(pq[:, sl], musrow[:], wcb[:, sl],
                                 start=False, stop=True)
            nc.vector.tensor_scalar(sbq[:], pq[:], rstd32[:], None,
                                    op0=ALU.mult)
        nc.sync.dma_start(aps["qkv_loc"].ap(), sbq[:])

        nc.gpsimd.collective_compute(
            "AllToAll", ALU.bypass, replica_groups=GROUPS,
            ins=[aps["qkv_loc"].ap()], outs=[aps["qkv_a2a"].ap()])

        # constants for the attention phase: load during the AllToAll
        eye4 = sg.tile([4, 4], F16)
        nc.sync.dma_start(eye4[:], C["eye4"].ap())
        eye8 = sg.tile([8, 8], F16)
        nc.sync.dma_start(eye8[:], C["eye8"].ap())
        rowsel = sg.tile([128, G], F16)
        nc.sync.dma_start(rowsel[:], C["rowsel"].ap())
        gcoln = sg.tile([G, 1], F32)
        nc.sync.dma_start(gcoln[:], C["gcoln"].ap())
        gcol = sg.tile([G, 1], F32)
        nc.sync.dma_start(gcol[:], C["gcol"].ap())
        gpH = sg.tile([G, 1], F32)
        nc.sync.dma_start(gpH[:], C["gpH"].ap())
        gmH = sg.tile([G, 1], F32)
        nc.sync.dma_start(gmH[:], C["gmH"].ap())
        gbc16 = sg.tile([128, G], F16)
        nc.sync.dma_start(gbc16[:], C["gbc16"].ap())
        sgn = sg.tile([128, 1], F32)
        nc.sync.dma_start(sgn[:], C["sgn"].ap())
        cmb = sg.tile([128, 36], F16)
        nc.sync.dma_start(cmb[:], C["cmb"].ap())

        # ---- attention (4 samples) ----
        a2a = aps["qkv_a2a"]
        QQ = sg.tile([128, W], F16, tag="QQ")
        h2sb = sg.tile([SPC, W], F16)
        numsb = sg.tile([SPC, W], F16)
        dinv = sg.tile([SPC, W], F32)
        with tc.tile_pool(name="v2pnd", bufs=1, space="PSUM") as pnd_pool:
            pnd = pnd_pool.tile([128, W], F32)
            with (
                tc.tile_pool(name="v2kv", bufs=2) as kv_pool,
                tc.tile_pool(name="v2qb", bufs=2) as qb_pool,
                tc.tile_pool(name="v2oh", bufs=2) as oh_pool,
                tc.tile_pool(name="v2mk", bufs=2) as mk_pool,
                tc.tile_pool(name="v2ptab", bufs=2, space="PSUM") as ptab_pool,
                tc.tile_pool(name="v2ptp", bufs=2, space="PSUM") as ptp_pool,
            ):
                engs = {"scalar": nc.scalar, "sync": nc.sync,
                        "gpsimd": nc.gpsimd}

                def _qq_dma(s):
                    engs[O["qq_eng"]].dma_start(
                        QQ[32 * s:32 * s + 32, :],
                        _ap(a2a, s * QKVW,
                            [[0, 32], [4 * QKVW, 8], [1, 256]]))

                def _finish_sample(s, fin):
                    ptab, oh = fin
                    sbtab = kv_pool.tile([4, G], F16, tag="sbtab")
                    nc.vector.tensor_copy(sbtab[:], ptab[:])
                    tabs6 = kv_pool.tile([G, 32], F16, tag="tabs")
                    nc.vector.memset(tabs6[:, 6:32], 0.0)
                    ptt = ptp_pool.tile([G, 4], F16, tag="ptp")
                    nc.tensor.transpose(ptt[:], sbtab[:], eye4[:])
                    nc.vector.tensor_copy(tabs6[:, 0:4], ptt[:])
                    ptt2 = ptp_pool.tile([G, 2], F32, tag="ptp")
                    nc.tensor.matmul(ptt2[:], rowsel[:], tabs6[:, 2:4],
                                     start=True, stop=True)
                    nc.vector.tensor_copy(tabs6[:, 4:6], ptt2[:])
                    for n in range(4):
                        sl = slice(n * 512, (n + 1) * 512)
                        nc.tensor.matmul(pnd[32 * s:32 * s + 32, sl],
                                         tabs6[:], oh[:, sl],
                                         start=True, stop=True,
                                         tile_position=(0, 32 * s))

                pending = None
                for s in range(SPC):
                    # q rows for the post-scale (broadcast to whole block)
                    if not O["qq_late"]:
                        _qq_dma(s)
                    # k,v transposed loads straight from DRAM (strided)
                    kT16 = kv_pool.tile([128, PCH], F16, tag="kT16")
                    vTt = kv_pool.tile([128, PCH], F16, tag="vT")
                    kv_row = kv_pool.tile([8, 512], F16, tag="kvrow")
                    engs[O["kv_eng"]].dma_start(
                        kv_row[:],
                        _ap(a2a, s * QKVW + FSL, [[4 * QKVW, 8], [1, 512]]))
                    for half in range(2):
                        ptk = ptp_pool.tile([128, 8], F16, tag="ptp")
                        nc.tensor.transpose(
                            ptk[:], kv_row[:, half * 128:(half + 1) * 128],
                            eye8[:])
                        nc.vector.tensor_copy(
                            kT16[:, half * 8:(half + 1) * 8], ptk[:])
                        ptv = ptp_pool.tile([128, 8], F16, tag="ptp")
                        nc.tensor.transpose(
                            ptv[:],
                            kv_row[:, 256 + half * 128:256 + (half + 1) * 128],
                            eye8[:])
                        nc.vector.tensor_copy(
                            vTt[:, half * 8:(half + 1) * 8], ptv[:])
                    kTt = kv_pool.tile([128, PCH], F32, tag="kT")
                    nc.vector.tensor_copy(kTt[:], kT16[:])

                    # u-vectors: e^k/64 * {v, 1}, e^-k/64 * {v, 1}
                    ek = kv_pool.tile([128, PCH], F16, tag="ek")
                    nc.scalar.activation(ek[:], kT16[:], ACTF.Exp,
                                         bias=mln64[:])
                    emk = kv_pool.tile([128, PCH], F16, tag="emk")
                    nc.scalar.activation(emk[:], kT16[:], ACTF.Exp,
                                         bias=mln64[:], scale=-1.0)
                    u = kv_pool.tile([128, PCH, 4], F16, tag="u")
                    nc.vector.tensor_mul(u[:, :, 0], ek[:], vTt[:])
                    nc.vector.tensor_copy(u[:, :, 1], ek[:])
                    nc.gpsimd.tensor_mul(u[:, :, 2], emk[:], vTt[:])
                    nc.gpsimd.tensor_copy(u[:, :, 3], emk[:])

                    # prefix masks for all chunks, then the table matmuls
                    ptab = ptab_pool.tile([4, G], F32, tag="ptab")
                    if O["masks"] == "prebuild":
                        mk_all = mk_pool.tile([128, PCH, G], F16, tag="mk")
                        for m in range(PCH):
                            eng = (nc.vector if (m % O["mask_dve_mod"] == 0)
                                   else nc.gpsimd)
                            eng.tensor_scalar(mk_all[:, m, :], gbc16[:],
                                              kTt[:, m:m + 1], None,
                                              op0=ALU.is_ge)
                        for m in range(PCH):
                            nc.tensor.matmul(ptab[:], u[:, m, :],
                                             mk_all[:, m, :],
                                             start=(m == 0),
                                             stop=(m == PCH - 1))
                    else:
                        for m in range(PCH):
                            mk = mk_pool.tile([128, G], F16, tag="mk")
                            eng = nc.vector if (m % 2 == 0) else nc.gpsimd
                            eng.tensor_scalar(mk[:], gbc16[:],
                                              kTt[:, m:m + 1], None,
                                              op0=ALU.is_ge)
                            nc.tensor.matmul(ptab[:], u[:, m, :], mk[:],
                                             start=(m == 0),
                                             stop=(m == PCH - 1))

                    # one-hot of nearest grid point (shared by all 4 rows)
                    qb = qb_pool.tile([128, W], F16, tag="qb")
                    engs[O["qb_eng"]].dma_start(
                        qb[:],
                        _ap(a2a, s * QKVW,
                            [[0, 128], [4 * QKVW, 8], [1, 256]]))
                    oh = oh_pool.tile([128, W], F16, tag="oh")
                    if O["t1_eng"] == "split3":
                        t1a = qb_pool.tile([128, W], F16, tag="t1")
                        t1b = qb_pool.tile([128, W], F16, tag="t1b")
                        nc.vector.tensor_scalar(t1a[:], qb[:], gpH[:], None,
                                                op0=ALU.is_le)
                        nc.vector.tensor_scalar(t1b[:], qb[:], gmH[:], None,
                                                op0=ALU.is_ge)
                        nc.vector.tensor_mul(oh[:], t1a[:], t1b[:])
                    else:
                        t1 = qb_pool.tile([128, W], F16, tag="t1")
                        nc.scalar.activation(t1[:], qb[:], ACTF.Abs,
                                             bias=gcoln[:])
                        nc.vector.tensor_scalar(oh[:], t1[:], HALF, None,
                                                op0=ALU.is_le)

                    if pending is not None:
                        _finish_sample(*pending)
                    pending = (s, (ptab, oh))
                _finish_sample(*pending)

            if O["qq_late"]:
                for s in range(SPC):
                    _qq_dma(s)
            # ---- post-scale + combine + divide (two halves, pipelined) ----
            E = sg.tile([128, W], F16, tag="E")
            nc.scalar.activation(E[:], QQ[:], ACTF.Exp, scale=sgn[:])
            SE = sg.tile([128, W], F16, tag="SE")
            with tc.tile_pool(name="v2p2", bufs=1, space="PSUM") as p2_pool:
                p2 = p2_pool.tile([36, W], F32)
                if O["tail"] == "quarters":
                    for n in range(4):
                        sl = slice(n * 512, (n + 1) * 512)
                        nc.vector.tensor_mul(SE[:, sl], pnd[:, sl],
                                             E[:, sl])
                        nc.tensor.matmul(p2[:, sl], cmb[:], SE[:, sl],
                                         start=True, stop=True)
                        nc.scalar.copy(numsb[:, sl], p2[0:4, sl])
                        nc.vector.reciprocal(dinv[:, sl], p2[32:36, sl])
                        nc.gpsimd.tensor_mul(h2sb[:, sl], numsb[:, sl],
                                             dinv[:, sl])
                        if not O["ag_t"]:
                            nc.sync.dma_start(aps["h2_loc"].ap()[:, sl],
                                              h2sb[:, sl])
                elif O["tail"] == "halves":
                    for hf in range(2):
                        hsl = slice(hf * 1024, (hf + 1) * 1024)
                        nc.vector.tensor_mul(SE[:, hsl], pnd[:, hsl],
                                             E[:, hsl])
                        for n in range(2 * hf, 2 * hf + 2):
                            sl = slice(n * 512, (n + 1) * 512)
                            nc.tensor.matmul(p2[:, sl], cmb[:], SE[:, sl],
                                             start=True, stop=True)
                        nc.scalar.copy(numsb[:, hsl], p2[0:4, hsl])
                        nc.vector.reciprocal(dinv[:, hsl], p2[32:36, hsl])
                        nc.gpsimd.tensor_mul(h2sb[:, hsl], numsb[:, hsl],
                                             dinv[:, hsl])
                        if not O["ag_t"]:
                            nc.sync.dma_start(aps["h2_loc"].ap()[:, hsl],
                                              h2sb[:, hsl])
                else:
                    nc.vector.tensor_mul(SE[:], pnd[:], E[:])
                    for n in range(4):
                        sl = slice(n * 512, (n + 1) * 512)
                        nc.tensor.matmul(p2[:, sl], cmb[:], SE[:, sl],
                                         start=True, stop=True)
                    nc.vector.reciprocal(dinv[:], p2[32:36, :])
                    nc.vector.tensor_mul(h2sb[:], p2[0:4, :], dinv[:])
                    nc.sync.dma_start(aps["h2_loc"].ap(), h2sb[:])

        if O["ag_t"]:
            # transpose h2 locally, AllGather along partitions
            h2tl = sg.tile([128, PCH * SPC], F16)
            with tc.tile_pool(name="v2pt1", bufs=2, space="PSUM") as pt1_pool:
                for m in range(PCH):
                    pt1 = pt1_pool.tile([128, SPC], F16)
                    nc.tensor.transpose(pt1[:],
                                        h2sb[:, m * 128:(m + 1) * 128],
                                        eye4[:])
                    nc.vector.tensor_copy(
                        h2tl[:, m * SPC:(m + 1) * SPC], pt1[:])
            nc.sync.dma_start(aps["h2t_loc"].ap(), h2tl[:])
            nc.gpsimd.collective_compute(
                "AllGather", ALU.bypass, replica_groups=GROUPS,
                ins=[aps["h2t_loc"].ap()], outs=[aps["h2t_gat"].ap()])
            h2T = sg.tile([128, PCH, B], F16)
            h2Tv = h2T[:].rearrange("p m (e sl) -> p m e sl", sl=SPC)
            nc.sync.dma_start(
                h2Tv[:],
                _ap(aps["h2t_gat"], 0,
                    [[PCH * SPC, 128], [SPC, PCH],
                     [128 * PCH * SPC, NCORES], [1, SPC]]))
        else:
            nc.gpsimd.collective_compute(
                "AllGather", ALU.bypass, replica_groups=GROUPS,
                ins=[aps["h2_loc"].ap()], outs=[aps["h2_gat"].ap()])
            h2T = sg.tile([128, PCH, B], F16)
            h2f = sg.tile([B, W], F16, tag="h2f")
            for qd in range(4):
                qsl = slice(qd * 512, (qd + 1) * 512)
                eng = (nc.sync, nc.scalar)[qd % 2]
                eng.dma_start(h2f[:, qsl], aps["h2_gat"].ap()[:, qsl])
            with tc.tile_pool(name="v2pt2", bufs=2, space="PSUM") as pt2_pool:
                for m in range(PCH):
                    pt2 = pt2_pool.tile([128, B], F16)
                    nc.tensor.transpose(pt2[:],
                                        h2f[:, m * 128:(m + 1) * 128],
                                        eye32[:])
                    nc.vector.tensor_copy(h2T[:, m, :], pt2[:])
        sbo = sg.tile([B, FSL], F32)
        with tc.tile_pool(name="v2po", bufs=1, space="PSUM") as po_pool:
            pout = po_pool.tile([B, FSL], F32)
            for m in range(PCH):
                nc.tensor.matmul(pout[:], h2T[:, m, :], wpst[:, m, :],
                                 start=(m == 0), stop=(m == PCH - 1))
            nc.vector.tensor_add(sbo[:], pout[:], xb[:])
        nc.sync.dma_start(aps["out"].ap(), sbo[:])


def make_in_maps_v2(inputs):
    x = np.ascontiguousarray(np.asarray(inputs["x"], np.float32))
    Wq = np.asarray(inputs["Wq"], np.float32)
    Wk = np.asarray(inputs["Wk"], np.float32)
    Wv = np.asarray(inputs["Wv"], np.float32)
    Wp = np.asarray(inputs["Wp"], np.float32)
    bq = np.asarray(inputs["bq"], np.float32)
    bk = np.asarray(inputs["bk"], np.float32)
    bv = np.asarray(inputs["bv"], np.float32)
    bp = np.asarray(inputs["bp"], np.float32)
    in_maps = []
    for c in range(NCORES):
        cs = slice(c * FSL, (c + 1) * FSL)
        in_maps.append({
            "x": x,
            "wqkv": np.ascontiguousarray(np.concatenate(
                [Wq[:, cs], Wk[:, cs], Wv[:, cs]], axis=1).astype(np.float16)),
            "wqcs": np.ascontiguousarray(-np.concatenate(
                [Wq[:, cs], Wk[:, cs], Wv[:, cs]],
                axis=1).sum(0).astype(np.float16).reshape(1, -1)),
            "bqkv": np.ascontiguousarray(np.concatenate(
                [bq[cs], bk[cs], bv[cs]]).astype(np.float16).reshape(1, -1)),
            "wp": np.ascontiguousarray(Wp[:, cs].astype(np.float16)),
            "bp": np.ascontiguousarray(bp[cs]),
            "xs": np.ascontiguousarray(x[:, cs]),
        })
    return in_maps


_BUILT = {}


def _get_nc(mode, skip_gb=False):
    key = (mode, skip_gb)
    if key not in _BUILT:
        if mode == "v2":
            _BUILT[key] = build_v2()
        else:
            _BUILT[key] = build(mode, skip_gb=skip_gb)
    return _BUILT[key]


def make_in_maps(inputs):
    x = np.ascontiguousarray(np.asarray(inputs["x"], np.float32))
    gamma = np.ascontiguousarray(np.asarray(inputs["gamma"], np.float32))
    beta = np.ascontiguousarray(np.asarray(inputs["beta"], np.float32))
    Wq = np.asarray(inputs["Wq"], np.float32)
    Wk = np.asarray(inputs["Wk"], np.float32)
    Wv = np.asarray(inputs["Wv"], np.float32)
    Wp = np.asarray(inputs["Wp"], np.float32)
    bq = np.asarray(inputs["bq"], np.float32)
    bk = np.asarray(inputs["bk"], np.float32)
    bv = np.asarray(inputs["bv"], np.float32)
    bp = np.asarray(inputs["bp"], np.float32)
    in_maps = []
    for c in range(NCORES):
        cs = slice(c * FSL, (c + 1) * FSL)
        in_maps.append({
            "x": x,
            "gamma": gamma,
            "beta": beta,
            "wqkv": np.ascontiguousarray(
                np.concatenate([Wq[:, cs], Wk[:, cs], Wv[:, cs]], axis=1)),
            "bqkv": np.ascontiguousarray(
                np.concatenate([bq[cs], bk[cs], bv[cs]])),
            "wp": np.ascontiguousarray(Wp[:, cs]),
            "bp": np.ascontiguousarray(bp[cs]),
            "xs": np.ascontiguousarray(x[:, cs]),
        })
    return in_maps


def kernel(**inputs):
    skip_gb = bool(
        np.all(np.asarray(inputs["gamma"], np.float32) == 1.0)
        and np.all(np.asarray(inputs["beta"], np.float32) == 0.0))
    mode = MODE
    if mode == "v2" and not skip_gb:
        mode = "binned"  # v2 assumes gamma=1, beta=0
    nc = _get_nc(mode, skip_gb)
    if mode == "v2":
        in_maps = make_in_maps_v2(inputs)
    else:
        in_maps = make_in_maps(inputs)
    res = run_bass_kernel_spmd(nc, in_maps, core_ids=list(range(NCORES)))
    out = np.concatenate([res.results[c]["out"] for c in range(NCORES)],
                         axis=1)
    return np.ascontiguousarray(out.astype(np.float32))



# revision 62
# speedup vs baseline: 1.7786x; 1.5098x over previous
"""Trainium2 Bass kernel for nn_AttnBlock_12704513262242.

Math (per sample b, W=2048 "positions" with scalar q/k values):
  h   = layernorm(x) * gamma + beta
  q,k,v = h @ W* + b*
  attn  = softmax(-|q_j - k_i|, over i)
  h2[j] = sum_i attn[j,i] * v[i]
  out   = x + h2 @ Wp + bp

Sharding: feature-parallel QKV/proj (each core owns a 256-col slice of all
four weight matrices, host-cast to fp16), AllToAll to redistribute q/k/v
sample-major, data-parallel attention (4 samples per core), AllGather of
h2, feature-sliced output projection.  Host concatenates 8 [32,256] slices.

Default mode "v2" (the fast path; "naive"/"binned" are older fallbacks):
  * LayerNorm is deferred through the QKV matmul: matmul raw x.T (starts
    immediately, warms the PE pstate), stats via a selector matmul on a
    [128,512] view + Newton rsqrt on DVE (no ACT table switches), then
    qkv = rstd*(x@W + mu*(-colsum W) + std*b) via one K=2 rank-2 matmul
    and a scaling psum->sbuf copy.
  * Binned softmin attention: exp(-|q-k|) = e^{k-q} (k<=q) + e^{q-k} (k>q).
    Per sample: single is_ge prefix masks (DVE/Pool) + indicator matmuls
    accumulate prefix tables A,C,PB,PD at G=128 grid points; totals TB,TD
    are extracted by a row-selector matmul and appended as constant table
    columns; one UNSCALED nearest-bin one-hot per query feeds one eval
    matmul producing all 6 rows; all 4 samples land in one [128,W] psum at
    32-aligned offsets.  Post-scaling by e^{-+q} ([128,W] exp with a
    per-partition sign column), a 0/+-1 combine matmul, reciprocal and
    multiply finish num/den -> h2.  e^k tables are scaled by 1/64 (cancels
    in num/den) for fp16 range safety.
  * DMA queueing matters on HW: broadcasts/loads on the two HWDGE queues
    (SP + ACT), never gpsimd/SWDGE; weight fp16 preloads are spread and
    overlapped; attention constants load during the AllToAll.
"""

import os
import sys

import numpy as np

for _p in ("/opt/trn_rl_repo", "/root/.axon_site/_ro/trn_rl_repo"):
    if os.path.isdir(_p) and _p not in sys.path:
        sys.path.insert(0, _p)

import concourse.bass as bass
import concourse.tile as tile
from concourse import bacc, mybir
from concourse.bass_utils import run_bass_kernel_spmd

F32 = mybir.dt.float32
F16 = mybir.dt.float16
ALU = mybir.AluOpType
ACTF = mybir.ActivationFunctionType

B = 32            # batch
W = 2048          # width (positions / features)
NCORES = 8
PCH = W // 128    # 16 partition chunks of the feature dim
FSL = W // NCORES  # 256 feature-slice per core
QKVW = 3 * FSL    # 768
SPC = B // NCORES  # 4 samples per core

G = 64            # grid bins for binned softmin (v2); err ~ (16/G)^2
LO, HI = -8.0, 8.0
DELTA = (HI - LO) / (G - 1)
HALF = DELTA / 2.0
EPS = 1e-6

MODE = os.environ.get("ATTN_MODE", "v2")
GROUPS = [list(range(NCORES))]


def _ap(tensor_handle, offset, ap):
    return bass.AP(tensor=tensor_handle, offset=offset, ap=ap)


def build(mode=None, reps=1, skip_gb=False, fake_cc=False,
          ohm_eng="dve", oh_bufs=2, mm16="dve", cc16=True, abl="full"):
    mode = mode or MODE
    nc = bacc.Bacc("TRN2", target_bir_lowering=False, debug=False,
                   num_devices=NCORES)

    x_t = nc.dram_tensor("x", [B, W], F32, kind="ExternalInput")
    gamma_t = nc.dram_tensor("gamma", [W], F32, kind="ExternalInput")
    beta_t = nc.dram_tensor("beta", [W], F32, kind="ExternalInput")
    wqkv_t = nc.dram_tensor("wqkv", [W, QKVW], F32, kind="ExternalInput")
    bqkv_t = nc.dram_tensor("bqkv", [QKVW], F32, kind="ExternalInput")
    wp_t = nc.dram_tensor("wp", [W, FSL], F32, kind="ExternalInput")
    bp_t = nc.dram_tensor("bp", [FSL], F32, kind="ExternalInput")
    xs_t = nc.dram_tensor("xs", [B, FSL], F32, kind="ExternalInput")
    out_t = nc.dram_tensor("out", [B, FSL], F32, kind="ExternalOutput")

    ccdt = F16 if cc16 else F32
    qkv_loc = nc.dram_tensor("qkv_loc", [B, QKVW], ccdt)
    qkv_a2a = nc.dram_tensor("qkv_a2a", [B, QKVW], ccdt)
    h2_loc = nc.dram_tensor("h2_loc", [SPC, W], ccdt)
    h2_gat = nc.dram_tensor("h2_gat", [B, W], ccdt, addr_space="Shared")

    c_eye32 = nc.inline_tensor(np.eye(32, dtype=np.float32), "c_eye32")
    c_eye8 = nc.inline_tensor(np.eye(8, dtype=np.float16), "c_eye8")
    c_eye8f = nc.inline_tensor(np.eye(8, dtype=np.float32), "c_eye8f")
    c_eye2 = nc.inline_tensor(np.eye(2, dtype=np.float32), "c_eye2")
    c_eye32_16 = nc.inline_tensor(np.eye(32, dtype=np.float16), "c_eye32_16")
    c_ones132 = nc.inline_tensor(np.ones((1, 32), np.float32), "c_ones132")
    gridv = np.linspace(LO, HI, G, dtype=np.float64).astype(np.float32)
    c_gcol = nc.inline_tensor(gridv.reshape(G, 1), "c_gcol")
    c_gcoln = nc.inline_tensor(-gridv.reshape(G, 1), "c_gcoln")
    c_grow = nc.inline_tensor(gridv.reshape(1, G), "c_grow")

    aps = dict(
        x=x_t.ap(), gamma=gamma_t.ap(), beta=beta_t.ap(),
        wqkv=wqkv_t.ap(), bqkv=bqkv_t.ap(), wp=wp_t.ap(), bp=bp_t.ap(),
        xs=xs_t.ap(), out=out_t.ap(),
        qkv_loc=qkv_loc.ap(), qkv_a2a=qkv_a2a.ap(),
        h2_loc=h2_loc.ap(), h2_gat=h2_gat.ap(),
        eye32=c_eye32.ap(), eye32_16=c_eye32_16.ap(),
        eye8=c_eye8.ap(), eye8f32=c_eye8f.ap(), eye2=c_eye2.ap(),
        ones132=c_ones132.ap(), gcol=c_gcol.ap(), gcoln=c_gcoln.ap(),
        grow=c_grow.ap(),
        a2a_tensor=qkv_a2a,
    )

    aps["fake_cc"] = fake_cc
    aps["ohm_eng"] = ohm_eng
    aps["oh_bufs"] = oh_bufs
    aps["mm16"] = mm16
    aps["cc16"] = cc16
    aps["abl"] = abl
    with tile.TileContext(nc) as tc:
        for _rep in range(reps):
            _build_tile(tc, aps, mode, skip_gb)

    nc.compile()
    return nc


def _build_tile(tc, aps, mode, skip_gb=False):
    nc = tc.nc

    with tc.tile_pool(name="singles", bufs=1) as singles:
        # ---- constants into SBUF ----
        eye32 = singles.tile([32, 32], F32)
        nc.sync.dma_start(eye32[:], aps["eye32"])
        eye32_16 = singles.tile([32, 32], F16)
        nc.sync.dma_start(eye32_16[:], aps["eye32_16"])
        eye8 = singles.tile([8, 8], F16 if aps["cc16"] else F32)
        nc.sync.dma_start(eye8[:], aps["eye8"]
                          if aps["cc16"] else aps["eye8f32"])
        eye2 = singles.tile([2, 2], F32)
        nc.sync.dma_start(eye2[:], aps["eye2"])
        ones132 = singles.tile([1, 32], F32)
        nc.sync.dma_start(ones132[:], aps["ones132"])
        gcol = singles.tile([G, 1], F32)
        nc.sync.dma_start(gcol[:], aps["gcol"])
        gcoln = singles.tile([G, 1], F32)
        nc.sync.dma_start(gcoln[:], aps["gcoln"])
        gbc = singles.tile([128, G], F32)
        nc.gpsimd.dma_start(gbc[:], aps["grow"].partition_broadcast(128))

        # ---- small weight bits ----
        bq32 = singles.tile([1, QKVW], F32)
        nc.sync.dma_start(bq32[:], aps["bqkv"].partition_broadcast(1))

        # residual + bp, exact fp32: xb = x_slice + bp
        xb = singles.tile([B, FSL], F32)
        bpb = singles.tile([B, FSL], F32)
        nc.gpsimd.dma_start(bpb[:], aps["bp"].partition_broadcast(B))
        xsl = singles.tile([B, FSL], F32)
        nc.sync.dma_start(xsl[:], aps["xs"])
        nc.vector.tensor_add(xb[:], xsl[:], bpb[:])

        # ---- layernorm (replicated, all 32 samples) ----
        sbx = singles.tile([B, W], F32, tag="bigio")
        nc.sync.dma_start(sbx[:], aps["x"])
        xg = sbx[:].rearrange("b (s f) -> b s f", s=4)  # 4 subgroups of 512
        stats = singles.tile([B, 4, 6], F32)
        for sg in range(4):
            nc.vector.bn_stats(stats[:, sg, :], xg[:, sg, :])
        mv = singles.tile([B, 2], F32)
        nc.vector.bn_aggr(mv[:], stats[:])
        eps_t = singles.tile([B, 1], F32)
        nc.vector.memset(eps_t[:], EPS)
        stdv = singles.tile([B, 1], F32)
        nc.scalar.activation(stdv[:], mv[:, 1:2], ACTF.Sqrt, bias=eps_t[:])
        rstd = singles.tile([B, 1], F32)
        nc.vector.reciprocal(rstd[:], stdv[:])
        h = singles.tile([B, W], F32)
        nc.vector.tensor_scalar(h[:], sbx[:], mv[:, 0:1], rstd[:],
                                op0=ALU.subtract, op1=ALU.mult)
        if not skip_gb:
            gb = singles.tile([B, W], F32, tag="gbb")
            nc.gpsimd.dma_start(gb[:], aps["gamma"].partition_broadcast(B))
            nc.vector.tensor_mul(h[:], h[:], gb[:])
            bb = singles.tile([B, W], F32, tag="gbb")
            nc.gpsimd.dma_start(bb[:], aps["beta"].partition_broadcast(B))
            nc.vector.tensor_add(h[:], h[:], bb[:])

        # ---- transpose h -> hT [128, PCH, 32] ----
        mm16 = aps["mm16"]
        wdt = F16 if mm16 != "off" else F32
        hT = singles.tile([128, PCH, B], wdt)
        with tc.tile_pool(name="ptr", bufs=2, space="PSUM") as ptr_pool:
            for ci in range(PCH):
                ptr = ptr_pool.tile([128, B], F32)
                nc.tensor.transpose(ptr[:], h[:, ci * 128:(ci + 1) * 128],
                                    eye32[:])
                nc.vector.tensor_copy(hT[:, ci, :], ptr[:])

        # ---- qkv matmul: [32, 768] = h @ wqkv + bqkv ----
        sbq = singles.tile([B, QKVW], F16 if aps["cc16"] else F32)
        with (
            tc.tile_pool(name="pq", bufs=1, space="PSUM") as pq_pool,
            tc.tile_pool(name="wst", bufs=4) as wst_pool,
        ):
            pq = pq_pool.tile([B, QKVW], F32)
            for ci in range(PCH):
                wch = wst_pool.tile([128, QKVW], F32, tag="wch")
                nc.sync.dma_start(wch[:],
                                  aps["wqkv"][ci * 128:(ci + 1) * 128, :])
                if mm16 == "off":
                    wmm = wch
                else:
                    wmm = wst_pool.tile([128, QKVW], F16, tag="wch16")
                    nc.vector.tensor_copy(wmm[:], wch[:])
                nc.tensor.matmul(pq[:, 0:512], hT[:, ci, :],
                                 wmm[:, 0:512],
                                 start=(ci == 0), stop=False)
                nc.tensor.matmul(pq[:, 512:QKVW], hT[:, ci, :],
                                 wmm[:, 512:QKVW],
                                 start=(ci == 0), stop=False)
            nc.tensor.matmul(pq[:, 0:512], ones132[:], bq32[:, 0:512],
                             start=False, stop=True)
            nc.tensor.matmul(pq[:, 512:QKVW], ones132[:], bq32[:, 512:QKVW],
                             start=False, stop=True)
            nc.vector.tensor_copy(sbq[:], pq[:])
        nc.sync.dma_start(aps["qkv_loc"], sbq[:])

        if aps.get("fake_cc"):
            nc.sync.dma_start(aps["qkv_a2a"], aps["qkv_loc"])
        else:
            nc.gpsimd.collective_compute(
                "AllToAll", ALU.bypass, replica_groups=GROUPS,
                ins=[aps["qkv_loc"]], outs=[aps["qkv_a2a"]])

        # ---- attention (4 samples) ----
        abl = aps["abl"]
        num_t = singles.tile([SPC, W], F32)
        den_t = singles.tile([SPC, W], F32)
        shared = dict(a2a=aps["a2a_tensor"], num=num_t, den=den_t,
                      eye8=eye8, eye2=eye2, gbc=gbc, gcol=gcol,
                      gcoln=gcoln, ohm_eng=aps["ohm_eng"],
                      oh_bufs=aps["oh_bufs"],
                      ccdt=F16 if aps["cc16"] else F32)
        if abl in ("no_attn", "qkv_only"):
            nc.vector.memset(num_t[:], 1.0)
            nc.vector.memset(den_t[:], 1.0)
        elif mode == "binned":
            _attn_binned(tc, shared)
        else:
            _attn_naive(tc, shared)

        dinv = singles.tile([SPC, W], F32)
        nc.vector.reciprocal(dinv[:], den_t[:])
        sbh2 = singles.tile([SPC, W], F16 if aps["cc16"] else F32)
        nc.vector.tensor_mul(sbh2[:], num_t[:], dinv[:])
        nc.sync.dma_start(aps["h2_loc"], sbh2[:])

        if abl in ("no_proj", "qkv_only"):
            nc.sync.dma_start(aps["out"], xb[:])
            return
        if aps.get("fake_cc"):
            nc.sync.dma_start(aps["h2_gat"][0:SPC, :], aps["h2_loc"])
        else:
            nc.gpsimd.collective_compute(
                "AllGather", ALU.bypass, replica_groups=GROUPS,
                ins=[aps["h2_loc"]], outs=[aps["h2_gat"]])

        # ---- output projection ----
        h2dt = F16 if aps["cc16"] else F32
        h2f = singles.tile([B, W], h2dt, tag="bigio2")
        nc.sync.dma_start(h2f[:], aps["h2_gat"])
        h2T = singles.tile([128, PCH, B], wdt)
        eyeh2 = eye32_16 if aps["cc16"] else eye32
        with tc.tile_pool(name="ptr2", bufs=2, space="PSUM") as ptr2_pool:
            for ci in range(PCH):
                ptr2 = ptr2_pool.tile([128, B], h2dt)
                nc.tensor.transpose(ptr2[:], h2f[:, ci * 128:(ci + 1) * 128],
                                    eyeh2[:])
                nc.vector.tensor_copy(h2T[:, ci, :], ptr2[:])

        sbo = singles.tile([B, FSL], F32)
        with (
            tc.tile_pool(name="pout", bufs=1, space="PSUM") as pout_pool,
            tc.tile_pool(name="wpst", bufs=4) as wpst_pool,
        ):
            pout = pout_pool.tile([B, FSL], F32)
            for ci in range(PCH):
                wpch = wpst_pool.tile([128, FSL], F32, tag="wpch")
                nc.sync.dma_start(wpch[:],
                                  aps["wp"][ci * 128:(ci + 1) * 128, :])
                if mm16 == "off":
                    wpmm = wpch
                else:
                    wpmm = wpst_pool.tile([128, FSL], F16, tag="wpch16")
                    nc.vector.tensor_copy(wpmm[:], wpch[:])
                nc.tensor.matmul(pout[:], h2T[:, ci, :], wpmm[:],
                                 start=(ci == 0), stop=(ci == PCH - 1))
            nc.vector.tensor_add(sbo[:], pout[:], xb[:])
        nc.sync.dma_start(aps["out"], sbo[:])


def _load_qkv_sample(nc, kv_pool, ptp_pool, shared, s):
    """Per-sample loads from the AllToAll result: broadcast q [128, W] and
    k/v transposed into [128, 16] (feature chunk m = half*8 + coreblk)."""
    a2a = shared["a2a"]
    eye8 = shared["eye8"]
    cdt = shared["ccdt"]
    dma = nc.sync.dma_start if cdt == F16 else nc.gpsimd.dma_start
    row_k = kv_pool.tile([8, 256], cdt, tag="krow")
    dma(row_k[:], _ap(a2a, s * QKVW + FSL, [[4 * QKVW, 8], [1, 256]]))
    row_v = kv_pool.tile([8, 256], cdt, tag="vrow")
    dma(row_v[:], _ap(a2a, s * QKVW + 2 * FSL, [[4 * QKVW, 8], [1, 256]]))
    kTt = kv_pool.tile([128, PCH], F32, tag="kT")
    vTt = kv_pool.tile([128, PCH], F32, tag="vT")
    for half in range(2):
        ptk = ptp_pool.tile([128, 8], cdt, tag="ptp")
        nc.tensor.transpose(ptk[:], row_k[:, half * 128:(half + 1) * 128],
                            eye8[:])
        nc.vector.tensor_copy(kTt[:, half * 8:(half + 1) * 8], ptk[:])
        ptv = ptp_pool.tile([128, 8], cdt, tag="ptp")
        nc.tensor.transpose(ptv[:], row_v[:, half * 128:(half + 1) * 128],
                            eye8[:])
        nc.vector.tensor_copy(vTt[:, half * 8:(half + 1) * 8], ptv[:])
    return kTt, vTt


def _q_broadcast(nc, pool, shared, s, clamp):
    qb = pool.tile([128, W], shared["ccdt"], tag="qb")
    src = _ap(shared["a2a"], s * QKVW, [[0, 128], [4 * QKVW, 8], [1, 256]])
    if shared["ccdt"] == F16:
        nc.sync.dma_start(qb[:], src)
    else:
        nc.gpsimd.dma_start(qb[:], src)
    if clamp:
        nc.vector.tensor_scalar(qb[:], qb[:], LO, HI,
                                op0=ALU.max, op1=ALU.min)
    return qb


def _attn_binned(tc, shared):
    nc = tc.nc
    gbc = shared["gbc"]
    gcoln = shared["gcoln"]
    eye2 = shared["eye2"]
    ohm_op = (nc.gpsimd.tensor_mul if shared["ohm_eng"] == "gpsimd"
              else nc.vector.tensor_mul)
    with (
        tc.tile_pool(name="akv", bufs=2) as kv_pool,
        tc.tile_pool(name="aqb", bufs=2) as qb_pool,
        tc.tile_pool(name="aoh", bufs=shared["oh_bufs"]) as oh_pool,
        tc.tile_pool(name="amk", bufs=3) as mk_pool,
        tc.tile_pool(name="atab", bufs=2) as tab_pool,
        tc.tile_pool(name="ptp", bufs=2, space="PSUM") as ptp_pool,
        tc.tile_pool(name="ptab", bufs=2, space="PSUM") as ptab_pool,
        tc.tile_pool(name="pnd", bufs=1, space="PSUM") as pnd_pool,
    ):
        for s in range(SPC):
            qb = _q_broadcast(nc, qb_pool, shared, s, clamp=False)
            kTt, vTt = _load_qkv_sample(nc, kv_pool, ptp_pool, shared, s)

            ek = kv_pool.tile([128, PCH], F32, tag="ek")
            nc.scalar.activation(ek[:], kTt[:], ACTF.Exp)
            emk = kv_pool.tile([128, PCH], F32, tag="emk")
            nc.scalar.activation(emk[:], kTt[:], ACTF.Exp, scale=-1.0)
            u = kv_pool.tile([128, PCH, 4], F16, tag="u")
            nc.vector.tensor_mul(u[:, :, 0], ek[:], vTt[:])
            nc.vector.tensor_copy(u[:, :, 1], ek[:])
            nc.vector.tensor_mul(u[:, :, 2], emk[:], vTt[:])
            nc.vector.tensor_copy(u[:, :, 3], emk[:])

            # cumulative tables at the G grid points: psum rows = u-type
            ptab = ptab_pool.tile([4, 2 * G], F32, tag="ptab")
            for m in range(PCH):
                mk = mk_pool.tile([128, 2 * G], F16, tag="mk")
                nc.vector.tensor_scalar(mk[:, 0:G], gbc[:],
                                        kTt[:, m:m + 1], None, op0=ALU.is_ge)
                nc.vector.tensor_scalar(mk[:, G:2 * G], gbc[:],
                                        kTt[:, m:m + 1], None, op0=ALU.is_lt)
                nc.tensor.matmul(ptab[:], u[:, m, :], mk[:],
                                 start=(m == 0), stop=(m == PCH - 1))
            # rows 0,1 x cols [0,G)  = A,C (prefix with e^k);
            # rows 2,3 x cols [G,2G) = B,D (suffix with e^-k)
            sbtab = tab_pool.tile([4, 2 * G], F32, tag="sbtab")
            nc.scalar.copy(sbtab[:], ptab[:])
            sbBD = tab_pool.tile([2, G], F32, tag="sbBD")
            nc.sync.dma_start(sbBD[:], sbtab[2:4, G:2 * G])
            tabs = tab_pool.tile([G, 4], F16, tag="tabs")
            ptt = ptp_pool.tile([G, 2], F32, tag="ptp")
            nc.tensor.transpose(ptt[:], sbtab[0:2, 0:G], eye2[:])
            nc.vector.tensor_copy(tabs[:, 0:2], ptt[:])
            ptt2 = ptp_pool.tile([G, 2], F32, tag="ptp")
            nc.tensor.transpose(ptt2[:], sbBD[:], eye2[:])
            nc.vector.tensor_copy(tabs[:, 2:4], ptt2[:])

            # one-hot of nearest grid point, pre-scaled by e^{-+q}
            t1 = qb_pool.tile([128, W], F32, tag="t1", bufs=2)
            nc.scalar.activation(t1[:], qb[:], ACTF.Abs, bias=gcoln[:])
            oh = oh_pool.tile([128, W], F16, tag="oh")
            nc.vector.tensor_scalar(oh[:], t1[:], HALF, None, op0=ALU.is_le)
            emq = oh_pool.tile([128, W], F16, tag="emq")
            nc.scalar.activation(emq[:], qb[:], ACTF.Exp, scale=-1.0)
            epq = oh_pool.tile([128, W], F16, tag="epq")
            nc.scalar.activation(epq[:], qb[:], ACTF.Exp, scale=1.0)
            ohm = oh_pool.tile([128, W], F16, tag="ohm")
            ohm_op(ohm[:], oh[:], emq[:])
            ohp = oh_pool.tile([128, W], F16, tag="ohp")
            ohm_op(ohp[:], oh[:], epq[:])

            pnd = pnd_pool.tile([2, W], F32, tag="pnd")
            for n in range(4):
                sl = slice(n * 512, (n + 1) * 512)
                nc.tensor.matmul(pnd[:, sl], tabs[:, 0:2], ohm[:, sl],
                                 start=True, stop=False)
                nc.tensor.matmul(pnd[:, sl], tabs[:, 2:4], ohp[:, sl],
                                 start=False, stop=True)
            ns_s = oh_pool.tile([2, W], F32, tag="ns")
            nc.scalar.copy(ns_s[:], pnd[:])
            nc.sync.dma_start(shared["num"][s:s + 1, :], ns_s[0:1, :])
            nc.sync.dma_start(shared["den"][s:s + 1, :], ns_s[1:2, :])


def _attn_naive(tc, shared):
    nc = tc.nc
    with (
        tc.tile_pool(name="akv", bufs=2) as kv_pool,
        tc.tile_pool(name="aqb", bufs=2) as qb_pool,
        tc.tile_pool(name="aab", bufs=2) as ab_pool,
        tc.tile_pool(name="apt", bufs=3) as pt_pool,
        tc.tile_pool(name="ptp", bufs=2, space="PSUM") as ptp_pool,
        tc.tile_pool(name="pnd", bufs=1, space="PSUM") as pnd_pool,
    ):
        for s in range(SPC):
            qb = _q_broadcast(nc, qb_pool, shared, s, clamp=False)
            kTt, vTt = _load_qkv_sample(nc, kv_pool, ptp_pool, shared, s)

            nk = kv_pool.tile([128, PCH], F32, tag="nk")
            nc.vector.tensor_scalar(nk[:], kTt[:], -1.0, None, op0=ALU.mult)
            u2 = kv_pool.tile([128, PCH, 2], F16, tag="u2")
            nc.vector.tensor_copy(u2[:, :, 0], vTt[:])
            nc.vector.memset(u2[:, :, 1], 1.0)

            pnd = pnd_pool.tile([2, W], F32, tag="pnd")
            for m in range(PCH):
                ab = ab_pool.tile([128, W], F32, tag="ab")
                nc.scalar.activation(ab[:], qb[:], ACTF.Abs,
                                     bias=nk[:, m:m + 1])
                pt = pt_pool.tile([128, W], F16, tag="pt")
                nc.scalar.activation(pt[:], ab[:], ACTF.Exp, scale=-1.0)
                for n in range(4):
                    sl = slice(n * 512, (n + 1) * 512)
                    nc.tensor.matmul(pnd[:, sl], u2[:, m, :], pt[:, sl],
                                     start=(m == 0), stop=(m == PCH - 1))
            ns_s = ab_pool.tile([2, W], F32, tag="ns")
            nc.scalar.copy(ns_s[:], pnd[:])
            nc.sync.dma_start(shared["num"][s:s + 1, :], ns_s[0:1, :])
            nc.sync.dma_start(shared["den"][s:s + 1, :], ns_s[1:2, :])


# ---------------------------------------------------------------------------
# v2: restructured kernel.
#   LN stats via matmul on [128,512] layout; fp16 weights (host-cast);
#   binned attention with prefix-only masks, shared unscaled one-hot,
#   batched post-scaling, all num/den in one PSUM tile; feature-par proj.
# ---------------------------------------------------------------------------

LN64 = float(np.log(64.0))


def _v2_consts(nc):
    G_ = G
    gridv = np.linspace(LO, HI, G_, dtype=np.float64).astype(np.float32)
    c = {}
    c["eye8"] = nc.inline_tensor(np.eye(8, dtype=np.float16), "v2_eye8")
    c["eye4"] = nc.inline_tensor(np.eye(4, dtype=np.float16), "v2_eye4")
    rs = np.zeros((128, G_), np.float16)
    rs[G_ - 1, :] = 1.0
    rs[127, :] = 1.0
    c["rowsel"] = nc.inline_tensor(rs, "v2_rowsel")
    c["gcoln2"] = nc.inline_tensor(
        np.concatenate([-gridv, -gridv]).reshape(128, 1), "v2_gcoln2")
    c["eye32"] = nc.inline_tensor(np.eye(32, dtype=np.float16), "v2_eye32")
    c["eye128"] = nc.inline_tensor(np.eye(128, dtype=np.float16), "v2_eye128")
    c["eye128f"] = nc.inline_tensor(np.eye(128, dtype=np.float32),
                                    "v2_eye128f")
    c["ones1"] = nc.inline_tensor(np.ones((1, 32), np.float16), "v2_ones1")
    c["gcoln"] = nc.inline_tensor(-gridv.reshape(G_, 1), "v2_gcoln")
    c["gcol"] = nc.inline_tensor(gridv.reshape(G_, 1), "v2_gcol")
    c["gpH"] = nc.inline_tensor(
        (gridv + np.float32(HALF)).reshape(G_, 1), "v2_gpH")
    c["gmH"] = nc.inline_tensor(
        (gridv - np.float32(HALF)).reshape(G_, 1), "v2_gmH")
    c["gbc16"] = nc.inline_tensor(
        np.tile(gridv.reshape(1, G_), (128, 1)).astype(np.float16), "v2_gbc16")
    # stats: selT[p, s] = 1 if p//4 == s  (x128 partition p = s*4 + c)
    selT = np.zeros((128, 32), np.float16)
    for p in range(128):
        selT[p, p // 4] = 1.0
    c["selT"] = nc.inline_tensor(selT, "v2_selT")
    # sign column for E = exp(sgn * q): rows (A,C -> -1), (B,D -> +1)
    sgn = np.ones((128, 1), np.float32)
    for s in range(4):
        sgn[32 * s + 0] = -1.0
        sgn[32 * s + 1] = -1.0
    c["sgn"] = nc.inline_tensor(sgn, "v2_sgn")
    # num_s = emq*A - epq*PB + epq*TB ; den_s = emq*C - epq*PD + epq*TD
    cmb = np.zeros((128, 36), np.float16)
    for s in range(4):
        cmb[32 * s + 0, s] = 1.0
        cmb[32 * s + 2, s] = -1.0
        cmb[32 * s + 4, s] = 1.0
        cmb[32 * s + 1, 32 + s] = 1.0
        cmb[32 * s + 3, 32 + s] = -1.0
        cmb[32 * s + 5, 32 + s] = 1.0
    c["cmb"] = nc.inline_tensor(cmb, "v2_cmb")
    return c


def build_v2(reps=1, opts=None):
    nc = bacc.Bacc("TRN2", target_bir_lowering=False, debug=False,
                   num_devices=NCORES)

    x_t = nc.dram_tensor("x", [B, W], F32, kind="ExternalInput")
    wqkv_t = nc.dram_tensor("wqkv", [W, QKVW], F16, kind="ExternalInput")
    wqcs_t = nc.dram_tensor("wqcs", [1, QKVW], F16, kind="ExternalInput")
    bqkv_t = nc.dram_tensor("bqkv", [1, QKVW], F16, kind="ExternalInput")
    wp_t = nc.dram_tensor("wp", [W, FSL], F16, kind="ExternalInput")
    bp_t = nc.dram_tensor("bp", [FSL], F32, kind="ExternalInput")
    xs_t = nc.dram_tensor("xs", [B, FSL], F32, kind="ExternalInput")
    out_t = nc.dram_tensor("out", [B, FSL], F32, kind="ExternalOutput")

    qkv_loc = nc.dram_tensor("qkv_loc", [B, QKVW], F16)
    qkv_a2a = nc.dram_tensor("qkv_a2a", [B, QKVW], F16)
    h2_loc = nc.dram_tensor("h2_loc", [SPC, W], F16)
    h2_gat = nc.dram_tensor("h2_gat", [B, W], F16, addr_space="Shared")
    h2t_loc = nc.dram_tensor("h2t_loc", [128, PCH * SPC], F16)
    h2t_gat = nc.dram_tensor("h2t_gat", [128 * NCORES, PCH * SPC], F16,
                             addr_space="Shared")

    consts = _v2_consts(nc)

    aps = dict(
        x=x_t, wqkv=wqkv_t, wqcs=wqcs_t, bqkv=bqkv_t, wp=wp_t, bp=bp_t,
        xs=xs_t,
        out=out_t, qkv_loc=qkv_loc, qkv_a2a=qkv_a2a,
        h2_loc=h2_loc, h2_gat=h2_gat, h2t_loc=h2t_loc, h2t_gat=h2t_gat,
        consts=consts,
    )
    aps["opts"] = dict(qb_eng="sync", qq_eng="sync", kv_eng="scalar",
                       masks="prebuild", tail="halves",
                       hT="pe", h2T="pe", stats_sq="pool", newton=True,
                       wdma="mix", qq_late=True, warm=0,
                       t1_eng="act", mask_dve_mod=4, ag_t=False)
    aps["opts"].update(opts or {})
    with tile.TileContext(nc) as tc:
        for _rep in range(reps):
            _build_tile_v2(tc, aps)
    nc.compile()
    return nc


def _build_tile_v2(tc, aps):
    nc = tc.nc
    C = aps["consts"]
    O = aps["opts"]

    with tc.tile_pool(name="v2s", bufs=1) as sg:
        # ---- input DMAs first (critical path) ----
        x128 = sg.tile([128, 512], F32)
        nc.sync.dma_start(x128[:], _ap(aps["x"], 0, [[512, 128], [1, 512]]))
        selT = sg.tile([128, 32], F16)
        nc.scalar.dma_start(selT[:], C["selT"].ap())
        eye128f = sg.tile([128, 128], F32)
        nc.scalar.dma_start(eye128f[:], C["eye128f"].ap())
        xsl = sg.tile([B, FSL], F32)
        nc.sync.dma_start(xsl[:], aps["xs"].ap())
        # LN inputs squared + f16 copy, ahead of any weight traffic
        xf = sg.tile([128, 1024], F16)
        nc.vector.tensor_copy(xf[:, 0:512], x128[:])
        if O["stats_sq"] == "pool":
            nc.gpsimd.tensor_mul(xf[:, 512:1024], x128[:], x128[:])
        else:
            nc.scalar.activation(xf[:, 512:1024], x128[:], ACTF.Square)
        wcb = sg.tile([2, QKVW], F16)
        nc.sync.dma_start(wcb[0:1, :], aps["wqcs"].ap())
        nc.sync.dma_start(wcb[1:2, :], aps["bqkv"].ap())
        eye32 = sg.tile([32, 32], F16)
        nc.scalar.dma_start(eye32[:], C["eye32"].ap())
        ones1 = sg.tile([1, 32], F16)
        nc.sync.dma_start(ones1[:], C["ones1"].ap())
        bq16 = sg.tile([1, QKVW], F16)
        nc.scalar.dma_start(bq16[:], aps["bqkv"].ap())
        mln64 = sg.tile([128, 1], F32)
        nc.vector.memset(mln64[:], -LN64)

        # ---- weights (fp16, preloaded; spread across queues) ----
        if O["wdma"] == "hwdge":
            wengs = (nc.scalar, nc.sync)
        else:
            wengs = (nc.scalar, nc.gpsimd)
        wst = sg.tile([128, PCH, QKVW], F16, tag="wst")
        for i in range(8):
            src = _ap(aps["wqkv"], i * 2 * 128 * QKVW,
                      [[QKVW, 128], [128 * QKVW, 2], [1, QKVW]])
            wengs[i % 2].dma_start(wst[:, 2 * i:2 * i + 2, :], src)
        wpst = sg.tile([128, PCH, FSL], F16, tag="wpst")
        for i in range(4):
            src = _ap(aps["wp"], i * 4 * 128 * FSL,
                      [[FSL, 128], [128 * FSL, 4], [1, FSL]])
            wengs[i % 2].dma_start(wpst[:, 4 * i:4 * i + 4, :], src)

        # residual + bp: xb = x_slice + bp
        xb = sg.tile([B, FSL], F32)
        bpb = sg.tile([B, FSL], F32)
        beng = nc.scalar if O["wdma"] == "hwdge" else nc.gpsimd
        beng.dma_start(bpb[:], aps["bp"].ap().partition_broadcast(B))
        nc.vector.tensor_add(xb[:], xsl[:], bpb[:])

        # ---- layernorm stats on [128, 512] layout ----
        with tc.tile_pool(name="v2pst", bufs=1, space="PSUM") as pst_pool:
            pstat = pst_pool.tile([32, 1024], F32)
            nc.tensor.matmul(pstat[:, 0:512], selT[:], xf[:, 0:512],
                             start=True, stop=True)
            nc.tensor.matmul(pstat[:, 512:1024], selT[:], xf[:, 512:1024],
                             start=True, stop=True)
            reds = sg.tile([32, 2], F32)
            nc.vector.tensor_reduce(reds[:, 0:1], pstat[:, 0:512],
                                    axis=mybir.AxisListType.X,
                                    op=ALU.add)
            nc.vector.tensor_reduce(reds[:, 1:2], pstat[:, 512:1024],
                                    axis=mybir.AxisListType.X,
                                    op=ALU.add)
        m2 = sg.tile([32, 2], F32)
        nc.vector.tensor_scalar(m2[:], reds[:], 1.0 / W, None, op0=ALU.mult)
        musq = sg.tile([32, 1], F32)
        nc.vector.tensor_mul(musq[:], m2[:, 0:1], m2[:, 0:1])
        var = sg.tile([32, 1], F32)
        nc.vector.tensor_sub(var[:], m2[:, 1:2], musq[:])
        # rstd via Newton rsqrt (var ~ 1, 3 iterations, all on DVE)
        y1 = sg.tile([32, 1], F32)
        nc.vector.tensor_scalar(y1[:], var[:], -0.5, 1.5,
                                op0=ALU.mult, op1=ALU.add)
        t_n = sg.tile([32, 4], F32, tag="newt")
        nc.vector.tensor_mul(t_n[:, 0:1], y1[:], y1[:])
        nc.vector.tensor_mul(t_n[:, 1:2], t_n[:, 0:1], var[:])
        nc.vector.tensor_scalar(t_n[:, 2:3], t_n[:, 1:2], -0.5, 1.5,
                                op0=ALU.mult, op1=ALU.add)
        y2 = sg.tile([32, 1], F32)
        nc.vector.tensor_mul(y2[:], y1[:], t_n[:, 2:3])
        t_m = sg.tile([32, 4], F32, tag="newt")
        nc.vector.tensor_mul(t_m[:, 0:1], y2[:], y2[:])
        nc.vector.tensor_mul(t_m[:, 1:2], t_m[:, 0:1], var[:])
        nc.vector.tensor_scalar(t_m[:, 2:3], t_m[:, 1:2], -0.5, 1.5,
                                op0=ALU.mult, op1=ALU.add)
        rstd32 = sg.tile([32, 1], F32)
        nc.vector.tensor_mul(rstd32[:], y2[:], t_m[:, 2:3])
        # mu_std cols (mu, std);  std = var * rstd = sqrt(var)
        mu_std = sg.tile([32, 2], F16)
        nc.vector.tensor_copy(mu_std[:, 0:1], m2[:, 0:1])
        nc.vector.tensor_mul(mu_std[:, 1:2], var[:], rstd32[:])
        musrow = sg.tile([2, 32], F16)

        # ---- transpose raw x (f32) -> xT [128, PCH, 32] (f16) ----
        hT = sg.tile([128, PCH, B], F16)
        hTv = hT[:].rearrange("p (c b) s -> p b c s", b=4)
        with tc.tile_pool(name="v2ptr", bufs=2, space="PSUM") as ptr_pool:
            for bb in range(4):
                ptr = ptr_pool.tile([128, 128], F32)
                nc.tensor.transpose(ptr[:],
                                    x128[:, bb * 128:(bb + 1) * 128],
                                    eye128f[:])
                ptr_r = ptr[:].rearrange("p (s c) -> p c s", c=4)
                nc.vector.tensor_copy(hTv[:, bb, :, :], ptr_r[:])

        # ---- qkv matmul on raw x; LN folded in afterwards ----
        # qkv = rstd*(xq + mu*(-S) + std*b)  with S = colsum(wqkv)
        sbq = sg.tile([B, QKVW], F16)
        with tc.tile_pool(name="v2pq", bufs=1, space="PSUM") as pq_pool:
            pq = pq_pool.tile([B, QKVW], F32)
            for m in range(PCH):
                nc.tensor.matmul(pq[:, 0:512], hT[:, m, :], wst[:, m, 0:512],
                                 start=(m == 0), stop=False)
                nc.tensor.matmul(pq[:, 512:QKVW], hT[:, m, :],
                                 wst[:, m, 512:QKVW],
                                 start=(m == 0), stop=False)
            with tc.tile_pool(name="v2pmu", bufs=1, space="PSUM") as pmu_pool:
                pmu = pmu_pool.tile([2, 32], F32)
                nc.tensor.matmul(pmu[:], mu_std[:], eye32[:],
                                 start=True, stop=True)
                nc.vector.tensor_copy(musrow[:], pmu[:])
            for sl in (slice(0, 512), slice(512, QKVW)):
                nc.tensor.matmul(pq[:, sl], musrow[:], wcb[:, sl],
                                 start=False, stop=True)
            nc.vector.tensor_scalar(sbq[:], pq[:], rstd32[:], None,
                                    op0=ALU.mult)
        nc.sync.dma_start(aps["qkv_loc"].ap(), sbq[:])

        nc.gpsimd.collective_compute(
            "AllToAll", ALU.bypass, replica_groups=GROUPS,
            ins=[aps["qkv_loc"].ap()], outs=[aps["qkv_a2a"].ap()])

        # constants for the attention phase: load during the AllToAll
        eye4 = sg.tile([4, 4], F16)
        nc.sync.dma_start(eye4[:], C["eye4"].ap())
        eye8 = sg.tile([8, 8], F16)
        nc.sync.dma_start(eye8[:], C["eye8"].ap())
        rowsel = sg.tile([128, G], F16)
        gcoln2 = sg.tile([128, 1], F32)
        nc.sync.dma_start(gcoln2[:], C["gcoln2"].ap())
        nc.sync.dma_start(rowsel[:], C["rowsel"].ap())
        gcoln = sg.tile([G, 1], F32)
        nc.sync.dma_start(gcoln[:], C["gcoln"].ap())
        gcol = sg.tile([G, 1], F32)
        nc.sync.dma_start(gcol[:], C["gcol"].ap())
        gpH = sg.tile([G, 1], F32)
        nc.sync.dma_start(gpH[:], C["gpH"].ap())
        gmH = sg.tile([G, 1], F32)
        nc.sync.dma_start(gmH[:], C["gmH"].ap())
        gbc16 = sg.tile([128, G], F16)
        nc.sync.dma_start(gbc16[:], C["gbc16"].ap())
        sgn = sg.tile([128, 1], F32)
        nc.sync.dma_start(sgn[:], C["sgn"].ap())
        cmb = sg.tile([128, 36], F16)
        nc.sync.dma_start(cmb[:], C["cmb"].ap())

        # ---- attention (4 samples) ----
        a2a = aps["qkv_a2a"]
        QQ = sg.tile([128, W], F16, tag="QQ")
        h2sb = sg.tile([SPC, W], F16)
        numsb = sg.tile([SPC, W], F16)
        dinv = sg.tile([SPC, W], F32)
        with tc.tile_pool(name="v2pnd", bufs=1, space="PSUM") as pnd_pool:
            pnd = pnd_pool.tile([128, W], F32)
            with (
                tc.tile_pool(name="v2kv", bufs=3) as kv_pool,
                tc.tile_pool(name="v2qb", bufs=3) as qb_pool,
                tc.tile_pool(name="v2oh", bufs=3) as oh_pool,
                tc.tile_pool(name="v2mk", bufs=3) as mk_pool,
                tc.tile_pool(name="v2ptab", bufs=2, space="PSUM") as ptab_pool,
                tc.tile_pool(name="v2ptp", bufs=2, space="PSUM") as ptp_pool,
            ):
                engs = {"scalar": nc.scalar, "sync": nc.sync,
                        "gpsimd": nc.gpsimd}

                def _qq_dma(s):
                    engs[O["qq_eng"]].dma_start(
                        QQ[32 * s:32 * s + 32, :],
                        _ap(a2a, s * QKVW,
                            [[0, 32], [4 * QKVW, 8], [1, 256]]))

                pair_tabs = {}

                def _finish_sample(s, fin):
                    ptab, oh2 = fin
                    qd = 64 * (s % 2)
                    qsl = slice(qd, qd + 64)
                    if s % 2 == 0:
                        tabs6 = kv_pool.tile([128, 32], F16, tag="tabs")
                        nc.vector.memset(tabs6[:, 6:32], 0.0)
                        pair_tabs[s // 2] = tabs6
                    else:
                        tabs6 = pair_tabs[s // 2]
                    sbtab = kv_pool.tile([4, G], F16, tag="sbtab")
                    nc.vector.tensor_copy(sbtab[:], ptab[:])
                    ptt = ptp_pool.tile([128, 4], F16, tag="ptp")
                    nc.tensor.transpose(ptt[qsl, :], sbtab[:], eye4[:])
                    nc.vector.tensor_copy(tabs6[qsl, 0:4], ptt[qsl, :])
                    ptt2 = ptp_pool.tile([128, 2], F32, tag="ptp")
                    nc.tensor.matmul(ptt2[qsl, :], rowsel[qsl, :],
                                     tabs6[qsl, 2:4],
                                     start=True, stop=True)
                    nc.vector.tensor_copy(tabs6[qsl, 4:6], ptt2[qsl, :])
                    for n in range(4):
                        sl = slice(n * 512, (n + 1) * 512)
                        nc.tensor.matmul(pnd[32 * s:32 * s + 32, sl],
                                         tabs6[qsl, :], oh2[qsl, sl],
                                         start=True, stop=True,
                                         tile_position=(qd, 32 * s))

                pending = None
                cur_oh = None
                for s in range(SPC):
                    # q rows for the post-scale (broadcast to whole block)
                    if not O["qq_late"]:
                        _qq_dma(s)
                    # k,v transposed loads straight from DRAM (strided)
                    kT16 = kv_pool.tile([128, PCH], F16, tag="kT16")
                    vTt = kv_pool.tile([128, PCH], F16, tag="vT")
                    kv_row = kv_pool.tile([8, 512], F16, tag="kvrow")
                    engs[O["kv_eng"]].dma_start(
                        kv_row[:],
                        _ap(a2a, s * QKVW + FSL, [[4 * QKVW, 8], [1, 512]]))
                    for half in range(2):
                        ptk = ptp_pool.tile([128, 8], F16, tag="ptp")
                        nc.tensor.transpose(
                            ptk[:], kv_row[:, half * 128:(half + 1) * 128],
                            eye8[:])
                        nc.vector.tensor_copy(
                            kT16[:, half * 8:(half + 1) * 8], ptk[:])
                        ptv = ptp_pool.tile([128, 8], F16, tag="ptp")
                        nc.tensor.transpose(
                            ptv[:],
                            kv_row[:, 256 + half * 128:256 + (half + 1) * 128],
                            eye8[:])
                        nc.vector.tensor_copy(
                            vTt[:, half * 8:(half + 1) * 8], ptv[:])
                    kTt = kv_pool.tile([128, PCH], F32, tag="kT")
                    nc.vector.tensor_copy(kTt[:], kT16[:])

                    # u-vectors: e^k/64 * {v, 1}, e^-k/64 * {v, 1}
                    ek = kv_pool.tile([128, PCH], F16, tag="ek")
                    nc.scalar.activation(ek[:], kT16[:], ACTF.Exp,
                                         bias=mln64[:])
                    emk = kv_pool.tile([128, PCH], F16, tag="emk")
                    nc.scalar.activation(emk[:], kT16[:], ACTF.Exp,
                                         bias=mln64[:], scale=-1.0)
                    u = kv_pool.tile([128, PCH, 4], F16, tag="u")
                    nc.vector.tensor_mul(u[:, :, 0], ek[:], vTt[:])
                    nc.vector.tensor_copy(u[:, :, 1], ek[:])
                    nc.gpsimd.tensor_mul(u[:, :, 2], emk[:], vTt[:])
                    nc.gpsimd.tensor_copy(u[:, :, 3], emk[:])

                    # prefix masks for all chunks, then the table matmuls
                    ptab = ptab_pool.tile([4, G], F32, tag="ptab")
                    if O["masks"] == "prebuild":
                        mk_all = mk_pool.tile([128, PCH, G], F16, tag="mk")
                        for m in range(PCH):
                            eng = (nc.vector if (m % O["mask_dve_mod"] == 0)
                                   else nc.gpsimd)
                            eng.tensor_scalar(mk_all[:, m, :], gbc16[:],
                                              kTt[:, m:m + 1], None,
                                              op0=ALU.is_ge)
                        for m in range(PCH):
                            nc.tensor.matmul(ptab[:], u[:, m, :],
                                             mk_all[:, m, :],
                                             start=(m == 0),
                                             stop=(m == PCH - 1))
                    else:
                        for m in range(PCH):
                            mk = mk_pool.tile([128, G], F16, tag="mk")
                            eng = nc.vector if (m % 2 == 0) else nc.gpsimd
                            eng.tensor_scalar(mk[:], gbc16[:],
                                              kTt[:, m:m + 1], None,
                                              op0=ALU.is_ge)
                            nc.tensor.matmul(ptab[:], u[:, m, :], mk[:],
                                             start=(m == 0),
                                             stop=(m == PCH - 1))

                    # one-hot of nearest grid point (shared by all 4 rows)
                    if s % 2 == 0:
                        qb = qb_pool.tile([128, W], F16, tag="qb")
                        for r in range(2):
                            engs[O["qb_eng"]].dma_start(
                                qb[64 * r:64 * r + 64, :],
                                _ap(a2a, (s + r) * QKVW,
                                    [[0, G], [4 * QKVW, 8], [1, 256]]))
                        t1 = qb_pool.tile([128, W], F16, tag="t1")
                        nc.scalar.activation(t1[:], qb[:], ACTF.Abs,
                                             bias=gcoln2[:])
                        cur_oh = oh_pool.tile([128, W], F16, tag="oh")
                        nc.vector.tensor_scalar(cur_oh[:], t1[:], HALF,
                                                None, op0=ALU.is_le)

                    if pending is not None:
                        _finish_sample(*pending)
                    pending = (s, (ptab, cur_oh))
                _finish_sample(*pending)

            if O["qq_late"]:
                for s in range(SPC):
                    _qq_dma(s)
            # ---- post-scale + combine + divide (two halves, pipelined) ----
            E = sg.tile([128, W], F16, tag="E")
            nc.scalar.activation(E[:], QQ[:], ACTF.Exp, scale=sgn[:])
            SE = sg.tile([128, W], F16, tag="SE")
            with tc.tile_pool(name="v2p2", bufs=1, space="PSUM") as p2_pool:
                p2 = p2_pool.tile([36, W], F32)
                if O["tail"] == "quarters":
                    for n in range(4):
                        sl = slice(n * 512, (n + 1) * 512)
                        nc.vector.tensor_mul(SE[:, sl], pnd[:, sl],
                                             E[:, sl])
                        nc.tensor.matmul(p2[:, sl], cmb[:], SE[:, sl],
                                         start=True, stop=True)
                        nc.scalar.copy(numsb[:, sl], p2[0:4, sl])
                        nc.vector.reciprocal(dinv[:, sl], p2[32:36, sl])
                        nc.gpsimd.tensor_mul(h2sb[:, sl], numsb[:, sl],
                                             dinv[:, sl])
                        if not O["ag_t"]:
                            nc.sync.dma_start(aps["h2_loc"].ap()[:, sl],
                                              h2sb[:, sl])
                elif O["tail"] == "halves":
                    for hf in range(2):
                        hsl = slice(hf * 1024, (hf + 1) * 1024)
                        nc.vector.tensor_mul(SE[:, hsl], pnd[:, hsl],
                                             E[:, hsl])
                        for n in range(2 * hf, 2 * hf + 2):
                            sl = slice(n * 512, (n + 1) * 512)
                            nc.tensor.matmul(p2[:, sl], cmb[:], SE[:, sl],
                                             start=True, stop=True)
                        nc.scalar.copy(numsb[:, hsl], p2[0:4, hsl])
                        nc.vector.reciprocal(dinv[:, hsl], p2[32:36, hsl])
                        nc.gpsimd.tensor_mul(h2sb[:, hsl], numsb[:, hsl],
                                             dinv[:, hsl])
                        if not O["ag_t"]:
                            nc.sync.dma_start(aps["h2_loc"].ap()[:, hsl],
                                              h2sb[:, hsl])
                else:
                    nc.vector.tensor_mul(SE[:], pnd[:], E[:])
                    for n in range(4):
                        sl = slice(n * 512, (n + 1) * 512)
                        nc.tensor.matmul(p2[:, sl], cmb[:], SE[:, sl],
                                         start=True, stop=True)
                    nc.vector.reciprocal(dinv[:], p2[32:36, :])
                    nc.vector.tensor_mul(h2sb[:], p2[0:4, :], dinv[:])
                    nc.sync.dma_start(aps["h2_loc"].ap(), h2sb[:])

        if O["ag_t"]:
            # transpose h2 locally, AllGather along partitions
            h2tl = sg.tile([128, PCH * SPC], F16)
            with tc.tile_pool(name="v2pt1", bufs=2, space="PSUM") as pt1_pool:
                for m in range(PCH):
                    pt1 = pt1_pool.tile([128, SPC], F16)
                    nc.tensor.transpose(pt1[:],
                                        h2sb[:, m * 128:(m + 1) * 128],
                                        eye4[:])
                    nc.vector.tensor_copy(
                        h2tl[:, m * SPC:(m + 1) * SPC], pt1[:])
            nc.sync.dma_start(aps["h2t_loc"].ap(), h2tl[:])
            nc.gpsimd.collective_compute(
                "AllGather", ALU.bypass, replica_groups=GROUPS,
                ins=[aps["h2t_loc"].ap()], outs=[aps["h2t_gat"].ap()])
            h2T = sg.tile([128, PCH, B], F16)
            h2Tv = h2T[:].rearrange("p m (e sl) -> p m e sl", sl=SPC)
            nc.sync.dma_start(
                h2Tv[:],
                _ap(aps["h2t_gat"], 0,
                    [[PCH * SPC, 128], [SPC, PCH],
                     [128 * PCH * SPC, NCORES], [1, SPC]]))
        else:
            nc.gpsimd.collective_compute(
                "AllGather", ALU.bypass, replica_groups=GROUPS,
                ins=[aps["h2_loc"].ap()], outs=[aps["h2_gat"].ap()])
            h2T = sg.tile([128, PCH, B], F16)
            h2f = sg.tile([B, W], F16, tag="h2f")
            for qd in range(4):
                qsl = slice(qd * 512, (qd + 1) * 512)
                eng = (nc.sync, nc.scalar)[qd % 2]
                eng.dma_start(h2f[:, qsl], aps["h2_gat"].ap()[:, qsl])
            with tc.tile_pool(name="v2pt2", bufs=2, space="PSUM") as pt2_pool:
                for m in range(PCH):
                    pt2 = pt2_pool.tile([128, B], F16)
                    nc.tensor.transpose(pt2[:],
                                        h2f[:, m * 128:(m + 1) * 128],
                                        eye32[:])
                    nc.vector.tensor_copy(h2T[:, m, :], pt2[:])
        sbo = sg.tile([B, FSL], F32)
        with tc.tile_pool(name="v2po", bufs=1, space="PSUM") as po_pool:
            pout = po_pool.tile([B, FSL], F32)
            for m in range(PCH):
                nc.tensor.matmul(pout[:], h2T[:, m, :], wpst[:, m, :],
                                 start=(m == 0), stop=(m == PCH - 1))
            nc.vector.tensor_add(sbo[:], pout[:], xb[:])
        nc.sync.dma_start(aps["out"].ap(), sbo[:])


def make_in_maps_v2(inputs):
    x = np.ascontiguousarray(np.asarray(inputs["x"], np.float32))
    Wq = np.asarray(inputs["Wq"], np.float32)
    Wk = np.asarray(inputs["Wk"], np.float32)
    Wv = np.asarray(inputs["Wv"], np.float32)
    Wp = np.asarray(inputs["Wp"], np.float32)
    bq = np.asarray(inputs["bq"], np.float32)
    bk = np.asarray(inputs["bk"], np.float32)
    bv = np.asarray(inputs["bv"], np.float32)
    bp = np.asarray(inputs["bp"], np.float32)
    in_maps = []
    for c in range(NCORES):
        cs = slice(c * FSL, (c + 1) * FSL)
        in_maps.append({
            "x": x,
            "wqkv": np.ascontiguousarray(np.concatenate(
                [Wq[:, cs], Wk[:, cs], Wv[:, cs]], axis=1).astype(np.float16)),
            "wqcs": np.ascontiguousarray(-np.concatenate(
                [Wq[:, cs], Wk[:, cs], Wv[:, cs]],
                axis=1).sum(0).astype(np.float16).reshape(1, -1)),
            "bqkv": np.ascontiguousarray(np.concatenate(
                [bq[cs], bk[cs], bv[cs]]).astype(np.float16).reshape(1, -1)),
            "wp": np.ascontiguousarray(Wp[:, cs].astype(np.float16)),
            "bp": np.ascontiguousarray(bp[cs]),
            "xs": np.ascontiguousarray(x[:, cs]),
        })
    return in_maps


_BUILT = {}


def _get_nc(mode, skip_gb=False):
    key = (mode, skip_gb)
    if key not in _BUILT:
        if mode == "v2":
            _BUILT[key] = build_v2()
        else:
            _BUILT[key] = build(mode, skip_gb=skip_gb)
    return _BUILT[key]


def make_in_maps(inputs):
    x = np.ascontiguousarray(np.asarray(inputs["x"], np.float32))
    gamma = np.ascontiguousarray(np.asarray(inputs["gamma"], np.float32))
    beta = np.ascontiguousarray(np.asarray(inputs["beta"], np.float32))
    Wq = np.asarray(inputs["Wq"], np.float32)
    Wk = np.asarray(inputs["Wk"], np.float32)
    Wv = np.asarray(inputs["Wv"], np.float32)
    Wp = np.asarray(inputs["Wp"], np.float32)
    bq = np.asarray(inputs["bq"], np.float32)
    bk = np.asarray(inputs["bk"], np.float32)
    bv = np.asarray(inputs["bv"], np.float32)
    bp = np.asarray(inputs["bp"], np.float32)
    in_maps = []
    for c in range(NCORES):
        cs = slice(c * FSL, (c + 1) * FSL)
        in_maps.append({
            "x": x,
            "gamma": gamma,
            "beta": beta,
            "wqkv": np.ascontiguousarray(
                np.concatenate([Wq[:, cs], Wk[:, cs], Wv[:, cs]], axis=1)),
            "bqkv": np.ascontiguousarray(
                np.concatenate([bq[cs], bk[cs], bv[cs]])),
            "wp": np.ascontiguousarray(Wp[:, cs]),
            "bp": np.ascontiguousarray(bp[cs]),
            "xs": np.ascontiguousarray(x[:, cs]),
        })
    return in_maps


def kernel(**inputs):
    skip_gb = bool(
        np.all(np.asarray(inputs["gamma"], np.float32) == 1.0)
        and np.all(np.asarray(inputs["beta"], np.float32) == 0.0))
    mode = MODE
    if mode == "v2" and not skip_gb:
        mode = "naive"  # v2/binned assume gamma=1, beta=0
    nc = _get_nc(mode, skip_gb)
    if mode == "v2":
        in_maps = make_in_maps_v2(inputs)
    else:
        in_maps = make_in_maps(inputs)
    res = run_bass_kernel_spmd(nc, in_maps, core_ids=list(range(NCORES)))
    out = np.concatenate([res.results[c]["out"] for c in range(NCORES)],
                         axis=1)
    return np.ascontiguousarray(out.astype(np.float32))

